# revision 1
# baseline (speedup 1.0000x reference)
"""Trainium2 Bass kernel for nn_MetricalGNN (2-layer hetero GraphSAGE).

Math (per layer, T=4 edge types):
    out = h @ mean_t(W_self[t]) + mean_t(b[t])
        + (1/T) * sum_t diag(1/max(cnt_t,1)) @ segsum_t(h[src]) @ W_neigh[t]
Layer 1 is followed by row-wise L2 normalize + ReLU.

Device strategy (8 cores, destination-sharded):
  - Each core owns a contiguous 6250-node destination range, processed in
    49 windows of 128 destinations.
  - Edges are sorted by (core, window, type) on the host and packed into
    128-edge chunks; all cores share one static chunk schedule (max over
    cores per (window, type) slot, padded).
  - Per chunk: one slice of a batched indirect DMA gathers the 128 source
    rows (fp16); DVE builds a scaled one-hot A[e,d] = scale_e * (iota==dst_e)
    in a single tensor_scalar op; the TensorEngine accumulates
    S_t^T[f,d] += M^T A into a per-type PSUM bank.  The per-edge scale folds
    in the 1/cnt mean, the 1/T type average, and padding (dst=200 -> 0 row).
  - The self term h_w @ W_self_avg is computed by the same machinery as a
    5th "type" whose edges are the window's own nodes with scale 1: its
    S^T is then exactly h_w^T, needing no separate transpose.
  - Stage 2 per window: copy the five S^T banks to SBUF (cast fp16) and run
    five matmuls out[d,fo] += S_t @ W_t into one PSUM bank, plus a K=1
    ones-row matmul adding the mean bias.
  - Layer-1 epilogue fuses square+row-sum (ACT accum), sqrt, reciprocal,
    and relu-with-per-row-scale (ACT) before storing h1 as fp16.
  - An AllGather exchanges the per-core h1 slices between layers; layer 2
    gathers from the concatenated [8*6272, 128] buffer via remapped indices.

Inputs are replicated/sharded on the host: x is pre-cast to fp16 and
replicated; per-core metadata tensors carry gather indices, window-local
destinations and scales; weights are packed to fp16 once.
"""

import numpy as np

N = 50000
E = 600000
F = 128
T = 4
C = 8                      # cores
NPC = N // C               # 6250 destinations per core
WPC = (NPC + 127) // 128   # 49 windows per core
NPC_PAD = WPC * 128        # 6272 rows per core slice
GB = 3                     # windows per batched gather instruction
PAD_DST = 200.0            # one-hot miss -> zero column


def _prep(x, W_self1, W_neigh1, b1, W_self2, W_neigh2, b2, edge_index, edge_type):
    src = np.asarray(edge_index[0], dtype=np.int64)
    dst = np.asarray(edge_index[1], dtype=np.int64)
    et = np.asarray(edge_type, dtype=np.int64)

    cnt = np.bincount(et * N + dst, minlength=T * N).reshape(T, N).astype(np.float32)
    scale_e = (0.25 / np.maximum(cnt[et, dst], 1.0)).astype(np.float32)

    core = dst // NPC
    win = (dst % NPC) // 128
    dloc = ((dst % NPC) % 128).astype(np.float32)

    order = np.lexsort((et, win, core))
    src_s, et_s, core_s, win_s = src[order], et[order], core[order], win[order]
    dloc_s, scale_s = dloc[order], scale_e[order]

    gkey = (core_s * WPC + win_s) * T + et_s
    counts = np.bincount(gkey, minlength=C * WPC * T).reshape(C, WPC, T)
    nchunk = np.maximum(1, -(-counts.max(axis=0) // 128))  # [WPC, T]

    # chunk layout: for w: [t0 chunks..., t1..., t2..., t3..., self]
    chunks_per_win = nchunk.sum(axis=1) + 1
    win_chunk_base = np.zeros(WPC, dtype=np.int64)
    win_chunk_base[1:] = np.cumsum(chunks_per_win)[:-1]
    NCH = int(chunks_per_win.sum())

    idx1 = np.zeros((C, NCH, 128), dtype=np.int32)
    idx2 = np.zeros((C, NCH, 128), dtype=np.int32)
    dstc = np.full((C, NCH, 128), PAD_DST, dtype=np.float32)
    sclc = np.zeros((C, NCH, 128), dtype=np.float32)

    glo = np.zeros(C * WPC * T + 1, dtype=np.int64)
    np.cumsum(np.bincount(gkey, minlength=C * WPC * T), out=glo[1:])

    permpos = (src // NPC) * NPC_PAD + (src % NPC)
    permpos_s = permpos[order].astype(np.int32)
    src_s32 = src_s.astype(np.int32)

    for c in range(C):
        for w in range(WPC):
            base = win_chunk_base[w]
            toff = 0
            for t in range(T):
                g = (c * WPC + w) * T + t
                lo, hi = glo[g], glo[g + 1]
                n = hi - lo
                s0 = (base + toff) * 128
                flat_i1 = idx1[c].reshape(-1)
                flat_i2 = idx2[c].reshape(-1)
                flat_d = dstc[c].reshape(-1)
                flat_s = sclc[c].reshape(-1)
                flat_i1[s0:s0 + n] = src_s32[lo:hi]
                flat_i2[s0:s0 + n] = permpos_s[lo:hi]
                flat_d[s0:s0 + n] = dloc_s[lo:hi]
                flat_s[s0:s0 + n] = scale_s[lo:hi]
                toff += nchunk[w, t]
            # self chunk
            sc = base + toff
            nd = min(128, NPC - w * 128)
            nodes = c * NPC + w * 128 + np.arange(nd)
            idx1[c, sc, :nd] = nodes.astype(np.int32)
            idx2[c, sc, :nd] = (c * NPC_PAD + w * 128 + np.arange(nd)).astype(np.int32)
            dstc[c, sc, :nd] = np.arange(nd, dtype=np.float32)
            sclc[c, sc, :nd] = 1.0

    # [C, NCH, 128] -> [C, 128, NCH] so column k holds chunk k's 128 rows
    idx1 = np.ascontiguousarray(idx1.transpose(0, 2, 1))
    idx2 = np.ascontiguousarray(idx2.transpose(0, 2, 1))
    dstc = np.ascontiguousarray(dstc.transpose(0, 2, 1))
    sclc = np.ascontiguousarray(sclc.transpose(0, 2, 1))

    wpack = np.empty((2 * (T + 1), F, F), dtype=np.float16)
    wpack[0:T] = np.asarray(W_neigh1, np.float32).astype(np.float16)
    wpack[T] = np.asarray(W_self1, np.float32).mean(axis=0).astype(np.float16)
    wpack[T + 1:2 * T + 1] = np.asarray(W_neigh2, np.float32).astype(np.float16)
    wpack[2 * T + 1] = np.asarray(W_self2, np.float32).mean(axis=0).astype(np.float16)

    bpack = np.stack([
        np.asarray(b1, np.float32).mean(axis=0),
        np.asarray(b2, np.float32).mean(axis=0),
    ]).astype(np.float16)

    x16 = np.asarray(x, np.float32).astype(np.float16)
    x16my = np.zeros((C, NPC_PAD, F), dtype=np.float16)
    for c in range(C):
        x16my[c, :NPC] = x16[c * NPC:(c + 1) * NPC]
    return idx1, idx2, dstc, sclc, wpack, bpack, x16, x16my, nchunk, NCH


def _legalize_sync_waits(nc, max_waits=1):
    """The walrus build in this container caps sync-wait commands per
    instruction; hoist excess waits onto NOPs inserted before the
    instruction on the same engine (sequencers execute in order)."""
    from concourse import mybir

    ctr = [0]
    for fn in nc.m.functions:
        for bb in fn.blocks:
            insts = bb.instructions
            if not any(
                i.sync_info is not None and len(i.sync_info.on_wait) > max_waits
                for i in insts
            ):
                continue
            out = []
            for inst in insts:
                si = inst.sync_info
                if si is not None and len(si.on_wait) > max_waits:
                    waits = list(si.on_wait)
                    keep = waits[-max_waits:]
                    hoist = waits[:-max_waits]
                    for i in range(0, len(hoist), max_waits):
                        nop = mybir.InstNoOp(
                            name=f"I-waitsplit-{ctr[0]}", ins=[], outs=[])
                        ctr[0] += 1
                        nop.engine = inst.engine
                        nop.sync_info = mybir.SyncInfo(
                            on_wait=hoist[i:i + max_waits], on_update=[])
                        out.append(nop)
                    inst.sync_info = mybir.SyncInfo(
                        on_wait=keep, on_update=list(si.on_update))
                out.append(inst)
            insts.clear()
            insts.extend(out)


def build_module(NCH, nchunk, legalize=True, n_cores=C):
    import concourse.bass as bass
    import concourse.tile as tile
    from concourse import mybir

    f16, f32, i32 = mybir.dt.float16, mybir.dt.float32, mybir.dt.int32
    Alu = mybir.AluOpType
    Act = mybir.ActivationFunctionType

    nc = bass.Bass(trn_type="TRN2")
    t_x16 = nc.dram_tensor("x16", [N, F], f16, kind="ExternalInput")
    t_x16my = nc.dram_tensor("x16my", [NPC_PAD, F], f16, kind="ExternalInput")
    t_idx1 = nc.dram_tensor("idx1", [128, NCH], i32, kind="ExternalInput")
    t_idx2 = nc.dram_tensor("idx2", [128, NCH], i32, kind="ExternalInput")
    t_dstc = nc.dram_tensor("dstc", [128, NCH], f32, kind="ExternalInput")
    t_sclc = nc.dram_tensor("sclc", [128, NCH], f32, kind="ExternalInput")
    t_wpack = nc.dram_tensor("wpack", [2 * (T + 1), F, F], f16, kind="ExternalInput")
    t_bpack = nc.dram_tensor("bpack", [2, F], f16, kind="ExternalInput")
    t_out = nc.dram_tensor("out", [NPC_PAD, F], f32, kind="ExternalOutput")

    chunks_per_win = nchunk.sum(axis=1) + 1
    win_chunk_base = np.zeros(WPC, dtype=np.int64)
    win_chunk_base[1:] = np.cumsum(chunks_per_win)[:-1]

    with tile.TileContext(nc, num_cores=n_cores) as tc:
        with tc.tile_pool(name="const", bufs=1) as cpool, \
             tc.tile_pool(name="gath", bufs=2) as gpool, \
             tc.tile_pool(name="onehot", bufs=6) as apool, \
             tc.tile_pool(name="stage2", bufs=2) as spool, \
             tc.tile_pool(name="epi", bufs=2) as epool, \
             tc.tile_pool(name="spsum", bufs=1, space="PSUM") as pspool, \
             tc.tile_pool(name="opsum", bufs=2, space="PSUM") as opool, \
             tc.tile_pool(name="dram", bufs=1, space="DRAM") as dpool:

            idx1_t = cpool.tile([128, NCH], i32)
            nc.sync.dma_start(out=idx1_t[:], in_=t_idx1[:])
            idx2_t = cpool.tile([128, NCH], i32)
            nc.sync.dma_start(out=idx2_t[:], in_=t_idx2[:])
            dstc_t = cpool.tile([128, NCH], f32)
            nc.sync.dma_start(out=dstc_t[:], in_=t_dstc[:])
            sclc_t = cpool.tile([128, NCH], f32)
            nc.sync.dma_start(out=sclc_t[:], in_=t_sclc[:])

            w_sb = cpool.tile([128, 2 * (T + 1) * F], f16)
            for k in range(2 * (T + 1)):
                nc.sync.dma_start(out=w_sb[:, k * F:(k + 1) * F], in_=t_wpack[k])
            b_sb = cpool.tile([1, 2 * F], f16)
            nc.sync.dma_start(out=b_sb[:, :F], in_=t_bpack[0:1, :])
            nc.sync.dma_start(out=b_sb[:, F:], in_=t_bpack[1:2, :])
            ones_sb = cpool.tile([1, 128], f16)
            nc.vector.memset(ones_sb[:], 1.0)
            eps_sb = cpool.tile([128, 1], f32)
            nc.vector.memset(eps_sb[:], 1e-24)
            zero_sb = cpool.tile([128, 1], f32)
            nc.vector.memset(zero_sb[:], 0.0)

            iota_i = cpool.tile([128, 128], i32)
            nc.gpsimd.iota(iota_i[:], pattern=[[1, 128]], base=0, channel_multiplier=0)
            iota_t = cpool.tile([128, 128], f32)
            nc.vector.tensor_copy(out=iota_t[:], in_=iota_i[:])

            h1_my = dpool.tile([NPC_PAD, F], f16)
            h1_all = dpool.tile([C * NPC_PAD, F], f16, addr_space="Shared")

            for layer in (0, 1):
                src_tbl = t_x16 if layer == 0 else h1_all
                self_tbl = t_x16my if layer == 0 else h1_my
                idx_t = idx1_t if layer == 0 else idx2_t
                wofs = layer * (T + 1) * F

                ss_all = epool.tile([128, WPC], f32, name=f"ss_all{layer}",
                                    tag=f"ss_all{layer}", bufs=1)
                o16 = []

                for w in range(WPC):
                    base = int(win_chunk_base[w])
                    s_ps = [pspool.tile([128, 128], f32, space="PSUM",
                                        name=f"s{t}", tag=f"s{t}")
                            for t in range(T + 1)]
                    ch = base
                    for t in range(T):
                        nk = int(nchunk[w, t])
                        for k in range(nk):
                            m_t = gpool.tile([128, F], f16, tag="m")
                            nc.gpsimd.indirect_dma_start(
                                out=m_t[:], out_offset=None, in_=src_tbl[:],
                                in_offset=bass.IndirectOffsetOnAxis(
                                    ap=idx_t[:, ch:ch + 1], axis=0))
                            a_t = apool.tile([128, 128], f16, tag="a")
                            nc.vector.tensor_scalar(
                                out=a_t[:], in0=iota_t[:],
                                scalar1=dstc_t[:, ch:ch + 1],
                                scalar2=sclc_t[:, ch:ch + 1],
                                op0=Alu.is_equal, op1=Alu.mult)
                            nc.tensor.matmul(
                                out=s_ps[t][:], lhsT=m_t[:], rhs=a_t[:],
                                start=(k == 0), stop=(k == nk - 1))
                            ch += 1
                    # self chunk: contiguous rows of my own slice
                    m_t = gpool.tile([128, F], f16, tag="m")
                    nc.sync.dma_start(
                        out=m_t[:], in_=self_tbl[w * 128:(w + 1) * 128, :])
                    a_t = apool.tile([128, 128], f16, tag="a")
                    nc.vector.tensor_scalar(
                        out=a_t[:], in0=iota_t[:],
                        scalar1=dstc_t[:, ch:ch + 1],
                        scalar2=sclc_t[:, ch:ch + 1],
                        op0=Alu.is_equal, op1=Alu.mult)
                    nc.tensor.matmul(
                        out=s_ps[T][:], lhsT=m_t[:], rhs=a_t[:],
                        start=True, stop=True)
                    ch += 1

                    # stage 2
                    o_ps = opool.tile([128, 128], f32, space="PSUM", tag="o")
                    s_sb = []
                    for t in range(T + 1):
                        st = spool.tile([128, 128], f16, tag=f"ssb{t}",
                                        name=f"ssb{t}")
                        if t < 3:
                            nc.vector.tensor_copy(out=st[:], in_=s_ps[t][:])
                        else:
                            nc.scalar.activation(out=st[:], in_=s_ps[t][:],
                                                 func=Act.Copy)
                        s_sb.append(st)
                    for t in range(T + 1):
                        nc.tensor.matmul(
                            out=o_ps[:], lhsT=s_sb[t][:],
                            rhs=w_sb[:, wofs + t * F: wofs + (t + 1) * F],
                            start=(t == 0), stop=False)
                    nc.tensor.matmul(
                        out=o_ps[:], lhsT=ones_sb[:],
                        rhs=b_sb[:, layer * F:(layer + 1) * F],
                        start=False, stop=True)

                    if layer == 0:
                        # stage to fp16 SBUF; square+reduce row sums now,
                        # sqrt/reciprocal batched once per layer
                        ow = epool.tile([128, 128], f16, name=f"o16_{w}",
                                        tag=f"o16_{w}", bufs=1)
                        nc.scalar.activation(out=ow[:], in_=o_ps[:],
                                             func=Act.Copy)
                        o16.append(ow)
                        sq = epool.tile([128, 128], f16, tag="sq")
                        nc.vector.tensor_tensor(
                            out=sq[:], in0=ow[:], in1=ow[:], op=Alu.mult)
                        nc.vector.tensor_reduce(
                            out=ss_all[:, w:w + 1], in_=sq[:],
                            axis=mybir.AxisListType.X, op=Alu.add)
                    else:
                        o_sb = epool.tile([128, 128], f32, tag="osb")
                        nc.scalar.activation(out=o_sb[:], in_=o_ps[:],
                                             func=Act.Copy)
                        nc.sync.dma_start(
                            out=t_out[w * 128:(w + 1) * 128, :], in_=o_sb[:])

                if layer == 0:
                    nrm_all = epool.tile([128, WPC], f32, name="nrm_all",
                                         tag="nrm_all", bufs=1)
                    nc.scalar.activation(out=nrm_all[:], in_=ss_all[:],
                                         func=Act.Sqrt, bias=eps_sb[:])
                    rn_all = epool.tile([128, WPC], f32, name="rn_all",
                                        tag="rn_all", bufs=1)
                    nc.vector.reciprocal(out=rn_all[:], in_=nrm_all[:])
                    for w in range(WPC):
                        h1_sb = epool.tile([128, 128], f16, tag="h1")
                        nc.vector.tensor_scalar(
                            out=h1_sb[:], in0=o16[w][:],
                            scalar1=rn_all[:, w:w + 1],
                            scalar2=zero_sb[:],
                            op0=Alu.mult, op1=Alu.max)
                        nc.sync.dma_start(
                            out=h1_my[w * 128:(w + 1) * 128, :], in_=h1_sb[:])
                    nc.gpsimd.collective_compute(
                        "AllGather",
                        mybir.AluOpType.bypass,
                        replica_groups=[list(range(n_cores))],
                        ins=[h1_my.opt()],
                        outs=[h1_all.opt()],
                    )

    if legalize:
        _legalize_sync_waits(nc)
    return nc


def kernel(**inputs):
    import sys
    if '/opt/trn_rl_repo' not in sys.path:
        sys.path.insert(0, '/opt/trn_rl_repo')

    idx1, idx2, dstc, sclc, wpack, bpack, x16, x16my, nchunk, NCH = _prep(
        inputs["x"], inputs["W_self1"], inputs["W_neigh1"], inputs["b1"],
        inputs["W_self2"], inputs["W_neigh2"], inputs["b2"],
        inputs["edge_index"], inputs["edge_type"])

    nc = build_module(NCH, nchunk, legalize=True, n_cores=C)

    from concourse.bass_utils import run_bass_kernel_spmd
    in_maps = [
        {"x16": x16, "x16my": x16my[c], "idx1": idx1[c], "idx2": idx2[c],
         "dstc": dstc[c], "sclc": sclc[c], "wpack": wpack, "bpack": bpack}
        for c in range(C)
    ]
    res = run_bass_kernel_spmd(nc, in_maps, core_ids=list(range(C)))

    out = np.empty((N, F), dtype=np.float32)
    for c in range(C):
        out[c * NPC:(c + 1) * NPC] = res.results[c]["out"][:NPC]
    return out



# revision 5
# speedup vs baseline: 3.1878x; 3.1878x over previous
"""Trainium2 Bass kernel for nn_MetricalGNN (2-layer hetero GraphSAGE).

Math (per layer, T=4 edge types):
    out = h @ mean_t(W_self[t]) + mean_t(b[t])
        + (1/T) * sum_t diag(1/max(cnt_t,1)) @ segsum_t(h[src]) @ W_neigh[t]
Layer 1 is followed by row-wise L2 normalize + ReLU.

Device strategy (8 cores, destination-sharded, 6250 nodes each in 49
windows of 128 destinations):
  - Edges are sorted by (core, window, type).  Within a window each type
    segment is padded to a 64-slot boundary (slot count = max over cores,
    so all cores share one compile-time schedule); the window's slot list
    is padded to a 128 multiple and split into 128-edge chunks.
  - One batched indirect DMA per 7 windows gathers all source rows of
    those windows' chunks (128 rows per chunk) into SBUF as fp16.  This
    amortizes the ~1us SWDGE descriptor-generation cost per instruction
    that dominated the per-chunk-gather baseline.
  - Per chunk, one tensor_scalar builds the scaled one-hot
    A[e,d] = scale_e * (iota==dst_e) (4x DVE mode; every 3rd chunk runs
    on gpsimd instead to balance engines).  scale folds the 1/cnt mean,
    the 1/T type average, and padding (dst=300 -> zero column).
  - The TensorEngine accumulates S_t^T[f,d] += M^T A into per-type
    column slices of a single [128,512] PSUM tile; a chunk straddling a
    type boundary issues one matmul per 32-aligned partition sub-range
    (legal sub-ranges: start 0/32/64/96 within the quadrant tree).
  - The self term h_w @ W_self_avg needs h_w^T as stationary operand: a
    PE transpose of the SBUF-resident window tile (x / h1 slices stay in
    SBUF all layer, partition-major) produces it in PSUM fp16 - no DMA.
  - Stage 2 per window (software-pipelined one window behind stage 1):
    one Act copy moves the [128,512] S^T block to SBUF fp16, one DVE
    copy moves the self block, then 6 matmuls (4 neighbor + self + K=1
    ones-row bias) accumulate out[d,fo] in one PSUM bank.
  - Layer-1 epilogue fuses square+row-sum (tensor_tensor_reduce), then
    batched sqrt/reciprocal and a per-window scale+relu into the
    resident h1 buffer; one DMA stores h1 (partition-major) and an
    AllGather shares it; layer 2 gathers from the concatenated
    [8*6272,128] table via host-remapped indices.
"""

import numpy as np

N = 50000
E = 600000
F = 128
T = 4
C = 8                      # cores
NPC = N // C               # 6250 destinations per core
WPC = (NPC + 127) // 128   # 49 windows per core
NPP = WPC * 128            # 6272 padded rows per core slice
GBW = 7                    # windows per batched gather instruction
PAD_DST = 300.0            # one-hot miss -> zero column


def _qblocks(lo, hi):
    """Split [lo,hi) (64-aligned) into partition ranges legal for PE
    operands (base partition must be 0, 32, or 64)."""
    out = []
    while lo < hi:
        if lo == 0:
            out.append((0, hi)); lo = hi
        elif lo == 64:
            out.append((64, hi)); lo = hi
        else:
            raise AssertionError(lo)
    return out


def _prep(x, W_self1, W_neigh1, b1, W_self2, W_neigh2, b2, edge_index, edge_type):
    src = np.asarray(edge_index[0], dtype=np.int64)
    dst = np.asarray(edge_index[1], dtype=np.int64)
    et = np.asarray(edge_type, dtype=np.int64)

    cnt = np.bincount(et * N + dst, minlength=T * N).reshape(T, N).astype(np.float32)
    scale_e = (0.25 / np.maximum(cnt[et, dst], 1.0)).astype(np.float32)

    core = dst // NPC
    loc = dst % NPC
    win = loc // 128
    dloc = (loc % 128).astype(np.float32)

    # per (core, window, type) counts; shared slot allocation = max over cores
    key = (core * WPC + win) * T + et
    n_cwt = np.bincount(key, minlength=C * WPC * T).reshape(C, WPC, T)
    S_wt = 64 * ((n_cwt.max(axis=0) + 63) // 64)          # [WPC, T]
    Wslots = S_wt.sum(axis=1)                              # [WPC]
    Kw = (Wslots + 127) // 128                             # chunks per window
    chbase = np.zeros(WPC + 1, dtype=np.int64)
    np.cumsum(Kw, out=chbase[1:])
    NCH = int(chbase[-1])
    tbase = np.zeros((WPC, T), dtype=np.int64)             # slot base of type seg
    tbase[:, 1:] = np.cumsum(S_wt, axis=1)[:, :-1]

    # per-edge slot assignment (order within a (c,w,t) group is arbitrary)
    order = np.lexsort((et, win, core))
    key_s = key[order]
    grp_start_of_key = np.searchsorted(key_s, np.arange(C * WPC * T))
    within = np.arange(E) - grp_start_of_key[key_s]
    sbase_flat = (chbase[:WPC, None] * 128 + tbase).reshape(-1)  # [WPC*T]
    g = sbase_flat[(win * T + et)[order]] + within         # global slot per edge
    p_slot = (g % 128).astype(np.int64)
    ch_slot = (g // 128).astype(np.int64)

    idx1 = np.zeros((C, 128, NCH), dtype=np.int32)
    idx2 = np.zeros((C, 128, NCH), dtype=np.int32)
    dstc = np.full((C, 128, NCH), PAD_DST, dtype=np.float32)
    sclc = np.zeros((C, 128, NCH), dtype=np.float32)

    src_s = src[order]
    csrc = src_s // NPC
    lsrc = src_s % NPC
    row2 = (csrc * NPP + (lsrc % 128) * WPC + lsrc // 128).astype(np.int32)
    core_s = core[order]
    idx1[core_s, p_slot, ch_slot] = src_s.astype(np.int32)
    idx2[core_s, p_slot, ch_slot] = row2
    dstc[core_s, p_slot, ch_slot] = dloc[order]
    sclc[core_s, p_slot, ch_slot] = scale_e[order]

    # compile-time matmul schedule per window: (ch, lo, hi, t, start, stop)
    sched = []
    for w in range(WPC):
        blocks = []
        for t in range(T):
            if S_wt[w, t] == 0:
                continue
            s0, s1 = int(tbase[w, t]), int(tbase[w, t] + S_wt[w, t])
            tblocks = []
            for ch in range(int(chbase[w]), int(chbase[w + 1])):
                c0 = (ch - chbase[w]) * 128
                lo, hi = max(s0, c0), min(s1, c0 + 128)
                if lo < hi:
                    for ql, qh in _qblocks(lo - c0, hi - c0):
                        tblocks.append([ch, ql, qh, t, False, False])
            tblocks[0][4] = True
            tblocks[-1][5] = True
            blocks.extend(tuple(b) for b in tblocks)
        sched.append(blocks)

    # weights / bias packs
    wpack = np.empty((2 * (T + 1), F, F), dtype=np.float16)
    wpack[0:T] = np.asarray(W_neigh1, np.float32).astype(np.float16)
    wpack[T] = np.asarray(W_self1, np.float32).mean(axis=0).astype(np.float16)
    wpack[T + 1:2 * T + 1] = np.asarray(W_neigh2, np.float32).astype(np.float16)
    wpack[2 * T + 1] = np.asarray(W_self2, np.float32).mean(axis=0).astype(np.float16)
    bpack = np.stack([
        np.asarray(b1, np.float32).mean(axis=0),
        np.asarray(b2, np.float32).mean(axis=0),
    ]).astype(np.float16)

    x16 = np.asarray(x, np.float32).astype(np.float16)
    xpad = np.zeros((C, NPP, F), dtype=np.float16)
    for c in range(C):
        xpad[c, :NPC] = x16[c * NPC:(c + 1) * NPC]
    # partition-major my-slice: xmyT[c, p, w*128+f] = x[c*NPC + w*128 + p, f]
    xmyT = np.ascontiguousarray(
        xpad.reshape(C, WPC, 128, F).transpose(0, 2, 1, 3).reshape(C, 128, NPP))

    return dict(idx1=idx1, idx2=idx2, dstc=dstc, sclc=sclc, wpack=wpack,
                bpack=bpack, x16=x16, xmyT=xmyT, sched=sched, Kw=Kw,
                chbase=chbase, NCH=NCH, S_wt=S_wt)


def _legalize_sync_waits(nc, max_waits=1):
    """The walrus build in this container caps sync-wait commands per
    instruction; hoist excess waits onto NOPs inserted before the
    instruction on the same engine (sequencers execute in order)."""
    from concourse import mybir

    ctr = [0]
    for fn in nc.m.functions:
        for bb in fn.blocks:
            insts = bb.instructions
            if not any(
                i.sync_info is not None and len(i.sync_info.on_wait) > max_waits
                for i in insts
            ):
                continue
            out = []
            for inst in insts:
                si = inst.sync_info
                if si is not None and len(si.on_wait) > max_waits:
                    waits = list(si.on_wait)
                    keep = waits[-max_waits:]
                    hoist = waits[:-max_waits]
                    for i in range(0, len(hoist), max_waits):
                        nop = mybir.InstNoOp(
                            name=f"I-waitsplit-{ctr[0]}", ins=[], outs=[])
                        ctr[0] += 1
                        nop.engine = inst.engine
                        nop.sync_info = mybir.SyncInfo(
                            on_wait=hoist[i:i + max_waits], on_update=[])
                        out.append(nop)
                    inst.sync_info = mybir.SyncInfo(
                        on_wait=keep, on_update=list(si.on_update))
                out.append(inst)
            insts.clear()
            insts.extend(out)


def build_module(prep, legalize=True, n_cores=C):
    import concourse.bass as bass
    import concourse.tile as tile
    from concourse import mybir

    f16, f32, i32 = mybir.dt.float16, mybir.dt.float32, mybir.dt.int32
    Alu = mybir.AluOpType
    Act = mybir.ActivationFunctionType

    NCH = prep["NCH"]
    Kw = prep["Kw"]
    chbase = prep["chbase"]
    sched = prep["sched"]

    nc = bass.Bass(trn_type="TRN2")
    t_x16 = nc.dram_tensor("x16", [N, F], f16, kind="ExternalInput")
    t_xmyT = nc.dram_tensor("xmyT", [128, NPP], f16, kind="ExternalInput")
    t_idx1 = nc.dram_tensor("idx1", [128, NCH], i32, kind="ExternalInput")
    t_idx2 = nc.dram_tensor("idx2", [128, NCH], i32, kind="ExternalInput")
    t_dstc = nc.dram_tensor("dstc", [128, NCH], f32, kind="ExternalInput")
    t_sclc = nc.dram_tensor("sclc", [128, NCH], f32, kind="ExternalInput")
    t_wpack = nc.dram_tensor("wpack", [2 * (T + 1), F, F], f16, kind="ExternalInput")
    t_bpack = nc.dram_tensor("bpack", [2, F], f16, kind="ExternalInput")
    t_out = nc.dram_tensor("out", [128, NPP], f16, kind="ExternalOutput")

    with tile.TileContext(nc, num_cores=n_cores) as tc:
        with tc.tile_pool(name="const", bufs=1) as cpool, \
             tc.tile_pool(name="gath", bufs=2) as gpool, \
             tc.tile_pool(name="onehot", bufs=8) as apool, \
             tc.tile_pool(name="stage2", bufs=2) as spool, \
             tc.tile_pool(name="epi", bufs=2) as epool, \
             tc.tile_pool(name="spsum", bufs=2, space="PSUM") as pspool, \
             tc.tile_pool(name="opsum", bufs=2, space="PSUM") as opool, \
             tc.tile_pool(name="dram", bufs=1, space="DRAM") as dpool:

            idx1_t = cpool.tile([128, NCH], i32)
            nc.sync.dma_start(out=idx1_t[:], in_=t_idx1[:])
            idx2_t = cpool.tile([128, NCH], i32)
            nc.sync.dma_start(out=idx2_t[:], in_=t_idx2[:])
            dstc_t = cpool.tile([128, NCH], f32)
            nc.sync.dma_start(out=dstc_t[:], in_=t_dstc[:])
            sclc_t = cpool.tile([128, NCH], f32)
            nc.sync.dma_start(out=sclc_t[:], in_=t_sclc[:])
            x_myb = cpool.tile([128, NPP], f16)
            nc.sync.dma_start(out=x_myb[:], in_=t_xmyT[:])

            w_sb = cpool.tile([128, 2 * (T + 1) * F], f16)
            for k in range(2 * (T + 1)):
                nc.sync.dma_start(out=w_sb[:, k * F:(k + 1) * F], in_=t_wpack[k])
            b_sb = cpool.tile([1, 2 * F], f16)
            nc.sync.dma_start(out=b_sb[:, :F], in_=t_bpack[0:1, :])
            nc.sync.dma_start(out=b_sb[:, F:], in_=t_bpack[1:2, :])
            ones_sb = cpool.tile([1, 128], f16)
            nc.vector.memset(ones_sb[:], 1.0)
            eps_sb = cpool.tile([128, 1], f32)
            nc.vector.memset(eps_sb[:], 1e-24)

            iota_i = cpool.tile([128, 128], i32)
            nc.gpsimd.iota(iota_i[:], pattern=[[1, 128]], base=0, channel_multiplier=0)
            iota_f = cpool.tile([128, 128], f16)
            nc.vector.tensor_copy(out=iota_f[:], in_=iota_i[:])
            iotac_i = cpool.tile([128, 1], i32)
            nc.gpsimd.iota(iotac_i[:], pattern=[[1, 1]], base=0, channel_multiplier=1)
            iotac_f = cpool.tile([128, 1], f32)
            nc.vector.tensor_copy(out=iotac_f[:], in_=iotac_i[:])
            ident = cpool.tile([128, 128], f16)
            nc.vector.tensor_scalar(out=ident[:], in0=iota_f[:],
                                    scalar1=iotac_f[:], scalar2=None,
                                    op0=Alu.is_equal)

            h1buf = cpool.tile([128, NPP], f16)
            obuf0 = cpool.tile([128, NPP], f16)
            obuf1 = cpool.tile([128, NPP], f16)
            ss = cpool.tile([128, WPC], f32)
            nrm = cpool.tile([128, WPC], f32)
            rn = cpool.tile([128, WPC], f32)

            h1_my = dpool.tile([128, NPP], f16)
            h1_all = dpool.tile([C * NPP, F], f16, addr_space="Shared")

            batches = [(w0, min(w0 + GBW, WPC)) for w0 in range(0, WPC, GBW)]

            for layer in (0, 1):
                idx_t = idx1_t if layer == 0 else idx2_t
                src_tbl = t_x16 if layer == 0 else h1_all
                selfsrc = x_myb if layer == 0 else h1buf
                obuf = obuf0 if layer == 0 else obuf1
                wofs = layer * (T + 1) * F

                abuild_ctr = 0
                pend = None  # (w, s_sb, s_sbT) awaiting stage 2

                def stage2(pw, ps_sb, ps_sbT):
                    o_ps = opool.tile([128, 128], f32, space="PSUM", tag="o")
                    first = True
                    for t in range(T):
                        if prep["S_wt"][pw, t] == 0:
                            continue
                        nc.tensor.matmul(
                            out=o_ps[:], lhsT=ps_sb[:, t * F:(t + 1) * F],
                            rhs=w_sb[:, wofs + t * F:wofs + (t + 1) * F],
                            start=first, stop=False)
                        first = False
                    nc.tensor.matmul(
                        out=o_ps[:], lhsT=ps_sbT[:],
                        rhs=w_sb[:, wofs + T * F:wofs + (T + 1) * F],
                        start=first, stop=False)
                    nc.tensor.matmul(
                        out=o_ps[:], lhsT=ones_sb[:],
                        rhs=b_sb[:, layer * F:(layer + 1) * F],
                        start=False, stop=True)
                    nc.scalar.activation(out=obuf[:, pw * 128:(pw + 1) * 128],
                                         in_=o_ps[:], func=Act.Copy)
                    if layer == 0:
                        sq = epool.tile([128, 128], f16, tag="sq")
                        nc.vector.tensor_tensor_reduce(
                            out=sq[:], in0=obuf[:, pw * 128:(pw + 1) * 128],
                            in1=obuf[:, pw * 128:(pw + 1) * 128],
                            scale=1.0, scalar=0.0, op0=Alu.mult, op1=Alu.add,
                            accum_out=ss[:, pw:pw + 1])

                for w0, w1 in batches:
                    ch0, ch1 = int(chbase[w0]), int(chbase[w1])
                    cw = ch1 - ch0
                    if cw > 0:
                        m_t = gpool.tile([128, cw * F], f16, tag="m")
                        nc.gpsimd.indirect_dma_start(
                            out=m_t[:], out_offset=None, in_=src_tbl[:],
                            in_offset=bass.IndirectOffsetOnAxis(
                                ap=idx_t[:, ch0:ch1], axis=0))
                    for w in range(w0, w1):
                        # stage 1: per-chunk scaled one-hot + matmul blocks
                        s_all = pspool.tile([128, T * 128], f32, space="PSUM",
                                            tag="sall")
                        s_self = pspool.tile([128, 128], f16, space="PSUM",
                                             tag="sself")
                        a_of_chunk = {}
                        for (ch, lo, hi, t, st, sp) in sched[w]:
                            if ch not in a_of_chunk:
                                a_t = apool.tile([128, 128], f16, tag="a")
                                eng = nc.gpsimd if abuild_ctr % 3 == 2 else nc.vector
                                abuild_ctr += 1
                                eng.tensor_scalar(
                                    out=a_t[:], in0=iota_f[:],
                                    scalar1=dstc_t[:, ch:ch + 1],
                                    scalar2=sclc_t[:, ch:ch + 1],
                                    op0=Alu.is_equal, op1=Alu.mult)
                                a_of_chunk[ch] = a_t
                            a_t = a_of_chunk[ch]
                            mc = (ch - ch0) * F
                            nc.tensor.matmul(
                                out=s_all[:, t * 128:(t + 1) * 128],
                                lhsT=m_t[lo:hi, mc:mc + F],
                                rhs=a_t[lo:hi, :], start=st, stop=sp)
                        nc.tensor.transpose(
                            out=s_self[:], in_=selfsrc[:, w * 128:(w + 1) * 128],
                            identity=ident[:])
                        # copies to SBUF for stage 2
                        s_sb = spool.tile([128, T * 128], f16, tag="ssb")
                        nc.scalar.activation(out=s_sb[:], in_=s_all[:],
                                             func=Act.Copy)
                        s_sbT = spool.tile([128, 128], f16, tag="ssbT")
                        nc.vector.tensor_copy(out=s_sbT[:], in_=s_self[:])
                        # stage 2 of the previous window (software pipeline)
                        if pend is not None:
                            stage2(*pend)
                        pend = (w, s_sb, s_sbT)
                if pend is not None:
                    stage2(*pend)

                if layer == 0:
                    nc.scalar.activation(out=nrm[:], in_=ss[:],
                                         func=Act.Sqrt, bias=eps_sb[:])
                    nc.vector.reciprocal(out=rn[:], in_=nrm[:])
                    for w in range(WPC):
                        nc.vector.tensor_scalar(
                            out=h1buf[:, w * 128:(w + 1) * 128],
                            in0=obuf0[:, w * 128:(w + 1) * 128],
                            scalar1=rn[:, w:w + 1], scalar2=0.0,
                            op0=Alu.mult, op1=Alu.max)
                    nc.sync.dma_start(out=h1_my[:], in_=h1buf[:])
                    nc.gpsimd.collective_compute(
                        "AllGather",
                        mybir.AluOpType.bypass,
                        replica_groups=[list(range(n_cores))],
                        ins=[h1_my.opt()],
                        outs=[h1_all.opt()],
                    )
                else:
                    nc.sync.dma_start(out=t_out[:], in_=obuf1[:])

    if legalize:
        _legalize_sync_waits(nc)
    return nc


def _unscramble(arr):
    # arr [128, NPP] fp16 partition-major -> [NPC, F] fp32 row-major
    return (arr.reshape(128, WPC, F).transpose(1, 0, 2)
            .reshape(NPP, F)[:NPC].astype(np.float32))


def kernel(**inputs):
    import sys
    if '/opt/trn_rl_repo' not in sys.path:
        sys.path.insert(0, '/opt/trn_rl_repo')

    prep = _prep(
        inputs["x"], inputs["W_self1"], inputs["W_neigh1"], inputs["b1"],
        inputs["W_self2"], inputs["W_neigh2"], inputs["b2"],
        inputs["edge_index"], inputs["edge_type"])

    nc = build_module(prep, legalize=True, n_cores=C)

    from concourse.bass_utils import run_bass_kernel_spmd
    in_maps = [
        {"x16": prep["x16"], "xmyT": prep["xmyT"][c], "idx1": prep["idx1"][c],
         "idx2": prep["idx2"][c], "dstc": prep["dstc"][c],
         "sclc": prep["sclc"][c], "wpack": prep["wpack"], "bpack": prep["bpack"]}
        for c in range(C)
    ]
    res = run_bass_kernel_spmd(nc, in_maps, core_ids=list(range(C)))

    out = np.empty((N, F), dtype=np.float32)
    for c in range(C):
        out[c * NPC:(c + 1) * NPC] = _unscramble(res.results[c]["out"])
    return out


# revision 12
# speedup vs baseline: 4.0025x; 1.2556x over previous
"""Trainium2 Bass kernel for nn_MetricalGNN (2-layer hetero GraphSAGE).

Math (per layer, T=4 edge types):
    out = h @ mean_t(W_self[t]) + mean_t(b[t])
        + (1/T) * sum_t diag(1/max(cnt_t,1)) @ segsum_t(h[src]) @ W_neigh[t]
Layer 1 is followed by row-wise L2 normalize + ReLU.

Device strategy (8 cores, destination-sharded, 6250 nodes each in 49
windows of 128 destinations):
  - Edges are sorted by (core, window, type).  Within a window each type
    segment is padded to a 64-slot boundary (slot count = max over cores,
    so all cores share one compile-time schedule); the window's slot list
    is padded to a 128 multiple and split into 128-edge chunks.
  - One batched indirect DMA per 7 windows gathers all source rows of
    those windows' chunks (128 rows per chunk) into SBUF as fp16.  This
    amortizes the ~1us SWDGE descriptor-generation cost per instruction
    that dominated the per-chunk-gather baseline.
  - Per chunk, one tensor_scalar builds the scaled one-hot
    A[e,d] = scale_e * (iota==dst_e) (4x DVE mode; every 3rd chunk runs
    on gpsimd instead to balance engines).  scale folds the 1/cnt mean,
    the 1/T type average, and padding (dst=300 -> zero column).
  - The TensorEngine accumulates S_t^T[f,d] += M^T A into per-type
    column slices of a single [128,512] PSUM tile; a chunk straddling a
    type boundary issues one matmul per 32-aligned partition sub-range
    (legal sub-ranges: start 0/32/64/96 within the quadrant tree).
  - The self term h_w @ W_self_avg needs h_w^T as stationary operand: a
    PE transpose of the SBUF-resident window tile (x / h1 slices stay in
    SBUF all layer, partition-major) produces it in PSUM fp16 - no DMA.
  - Stage 2 per window (software-pipelined one window behind stage 1):
    one Act copy moves the [128,512] S^T block to SBUF fp16, one DVE
    copy moves the self block, then 6 matmuls (4 neighbor + self + K=1
    ones-row bias) accumulate out[d,fo] in one PSUM bank.
  - Layer-1 epilogue fuses square+row-sum (tensor_tensor_reduce), then
    batched sqrt/reciprocal and a per-window scale+relu into the
    resident h1 buffer; one DMA stores h1 (partition-major) and an
    AllGather shares it; layer 2 gathers from the concatenated
    [8*6272,128] table via host-remapped indices.
"""

import numpy as np

N = 50000
E = 600000
F = 128
T = 4
C = 8                      # cores
NPC = N // C               # 6250 destinations per core
WPC = (NPC + 127) // 128   # 49 windows per core
NPP = WPC * 128            # 6272 padded rows per core slice
GBW = 7                    # windows per batched gather instruction
PAD_DST = 300.0            # one-hot miss -> zero column


def _qblocks(lo, hi):
    """Split [lo,hi) (64-aligned) into partition ranges legal for PE
    operands (base partition must be 0, 32, or 64)."""
    out = []
    while lo < hi:
        if lo == 0:
            out.append((0, hi)); lo = hi
        elif lo == 64:
            out.append((64, hi)); lo = hi
        else:
            raise AssertionError(lo)
    return out


def _prep(x, W_self1, W_neigh1, b1, W_self2, W_neigh2, b2, edge_index, edge_type):
    src = np.asarray(edge_index[0], dtype=np.int64)
    dst = np.asarray(edge_index[1], dtype=np.int64)
    et = np.asarray(edge_type, dtype=np.int64)

    cnt = np.bincount(et * N + dst, minlength=T * N).reshape(T, N).astype(np.float32)
    scale_e = (0.25 / np.maximum(cnt[et, dst], 1.0)).astype(np.float32)

    core = dst // NPC
    loc = dst % NPC
    win = loc // 128
    dloc = (loc % 128).astype(np.float32)

    # per (core, window, type) counts; shared slot allocation = max over cores
    key = (core * WPC + win) * T + et
    n_cwt = np.bincount(key, minlength=C * WPC * T).reshape(C, WPC, T)
    S_wt = 64 * ((n_cwt.max(axis=0) + 63) // 64)          # [WPC, T]
    Wslots = S_wt.sum(axis=1)                              # [WPC]
    Kw = (Wslots + 127) // 128                             # chunks per window
    chbase = np.zeros(WPC + 1, dtype=np.int64)
    np.cumsum(Kw, out=chbase[1:])
    NCH = int(chbase[-1])
    tbase = np.zeros((WPC, T), dtype=np.int64)             # slot base of type seg
    tbase[:, 1:] = np.cumsum(S_wt, axis=1)[:, :-1]

    # per-edge slot assignment (order within a (c,w,t) group is arbitrary)
    order = np.lexsort((et, win, core))
    key_s = key[order]
    grp_start_of_key = np.searchsorted(key_s, np.arange(C * WPC * T))
    within = np.arange(E) - grp_start_of_key[key_s]
    sbase_flat = (chbase[:WPC, None] * 128 + tbase).reshape(-1)  # [WPC*T]
    g = sbase_flat[(win * T + et)[order]] + within         # global slot per edge
    p_slot = (g % 128).astype(np.int64)
    ch_slot = (g // 128).astype(np.int64)

    idx1 = np.zeros((C, 128, NCH), dtype=np.int32)
    idx2 = np.zeros((C, 128, NCH), dtype=np.int32)
    dstc = np.full((C, 128, NCH), PAD_DST, dtype=np.float32)
    sclc = np.zeros((C, 128, NCH), dtype=np.float32)

    # indices premultiplied by F: gather tables are addressed as flat 1-D
    # arrays so each descriptor covers a whole per-partition run (>=512B,
    # avoiding the sub-512B descriptor latency penalty)
    src_s = src[order]
    csrc = src_s // NPC
    lsrc = src_s % NPC
    row2 = ((csrc * NPP + (lsrc % 128) * WPC + lsrc // 128) * F).astype(np.int32)
    core_s = core[order]
    idx1[core_s, p_slot, ch_slot] = (src_s * F).astype(np.int32)
    idx2[core_s, p_slot, ch_slot] = row2
    dstc[core_s, p_slot, ch_slot] = dloc[order]
    sclc[core_s, p_slot, ch_slot] = scale_e[order]

    # compile-time matmul schedule per window: (ch, lo, hi, t, start, stop)
    sched = []
    for w in range(WPC):
        blocks = []
        for t in range(T):
            if S_wt[w, t] == 0:
                continue
            s0, s1 = int(tbase[w, t]), int(tbase[w, t] + S_wt[w, t])
            tblocks = []
            for ch in range(int(chbase[w]), int(chbase[w + 1])):
                c0 = (ch - chbase[w]) * 128
                lo, hi = max(s0, c0), min(s1, c0 + 128)
                if lo < hi:
                    for ql, qh in _qblocks(lo - c0, hi - c0):
                        tblocks.append([ch, ql, qh, t, False, False])
            tblocks[0][4] = True
            tblocks[-1][5] = True
            blocks.extend(tuple(b) for b in tblocks)
        sched.append(blocks)

    # weights / bias packs
    wpack = np.empty((2 * (T + 1), F, F), dtype=np.float16)
    wpack[0:T] = np.asarray(W_neigh1, np.float32).astype(np.float16)
    wpack[T] = np.asarray(W_self1, np.float32).mean(axis=0).astype(np.float16)
    wpack[T + 1:2 * T + 1] = np.asarray(W_neigh2, np.float32).astype(np.float16)
    wpack[2 * T + 1] = np.asarray(W_self2, np.float32).mean(axis=0).astype(np.float16)
    bpack = np.stack([
        np.asarray(b1, np.float32).mean(axis=0),
        np.asarray(b2, np.float32).mean(axis=0),
    ]).astype(np.float16)

    x16 = np.asarray(x, np.float32).astype(np.float16)
    xpad = np.zeros((C, NPP, F), dtype=np.float16)
    for c in range(C):
        xpad[c, :NPC] = x16[c * NPC:(c + 1) * NPC]
    # partition-major my-slice: xmyT[c, p, w*128+f] = x[c*NPC + w*128 + p, f]
    xmyT = np.ascontiguousarray(
        xpad.reshape(C, WPC, 128, F).transpose(0, 2, 1, 3).reshape(C, 128, NPP))

    return dict(idx1=idx1, idx2=idx2, dstc=dstc, sclc=sclc, wpack=wpack,
                bpack=bpack, x16=x16, xmyT=xmyT, sched=sched, Kw=Kw,
                chbase=chbase, NCH=NCH, S_wt=S_wt)


def _legalize_sync_waits(nc, max_waits=1):
    """The walrus build in this container caps sync-wait commands per
    instruction; hoist excess waits onto NOPs inserted before the
    instruction on the same engine (sequencers execute in order)."""
    from concourse import mybir

    ctr = [0]
    for fn in nc.m.functions:
        for bb in fn.blocks:
            insts = bb.instructions
            if not any(
                i.sync_info is not None and len(i.sync_info.on_wait) > max_waits
                for i in insts
            ):
                continue
            out = []
            for inst in insts:
                si = inst.sync_info
                if si is not None and len(si.on_wait) > max_waits:
                    waits = list(si.on_wait)
                    keep = waits[-max_waits:]
                    hoist = waits[:-max_waits]
                    for i in range(0, len(hoist), max_waits):
                        nop = mybir.InstNoOp(
                            name=f"I-waitsplit-{ctr[0]}", ins=[], outs=[])
                        ctr[0] += 1
                        nop.engine = inst.engine
                        nop.sync_info = mybir.SyncInfo(
                            on_wait=hoist[i:i + max_waits], on_update=[])
                        out.append(nop)
                    inst.sync_info = mybir.SyncInfo(
                        on_wait=keep, on_update=list(si.on_update))
                out.append(inst)
            insts.clear()
            insts.extend(out)


def build_module(prep, legalize=True, n_cores=C):
    import concourse.bass as bass
    import concourse.tile as tile
    from concourse import mybir

    f16, f32, i32 = mybir.dt.float16, mybir.dt.float32, mybir.dt.int32
    Alu = mybir.AluOpType
    Act = mybir.ActivationFunctionType

    NCH = prep["NCH"]
    Kw = prep["Kw"]
    chbase = prep["chbase"]
    sched = prep["sched"]

    nc = bass.Bass(trn_type="TRN2")
    t_x16 = nc.dram_tensor("x16", [N, F], f16, kind="ExternalInput")
    t_xmyT = nc.dram_tensor("xmyT", [128, NPP], f16, kind="ExternalInput")
    t_idx1 = nc.dram_tensor("idx1", [128, NCH], i32, kind="ExternalInput")
    t_idx2 = nc.dram_tensor("idx2", [128, NCH], i32, kind="ExternalInput")
    t_dstc = nc.dram_tensor("dstc", [128, NCH], f32, kind="ExternalInput")
    t_sclc = nc.dram_tensor("sclc", [128, NCH], f32, kind="ExternalInput")
    t_wpack = nc.dram_tensor("wpack", [2 * (T + 1), F, F], f16, kind="ExternalInput")
    t_bpack = nc.dram_tensor("bpack", [2, F], f16, kind="ExternalInput")
    t_out = nc.dram_tensor("out", [128, NPP], f16, kind="ExternalOutput")

    with tile.TileContext(nc, num_cores=n_cores) as tc:
        with tc.tile_pool(name="const", bufs=1) as cpool, \
             tc.tile_pool(name="gath", bufs=2) as gpool, \
             tc.tile_pool(name="onehot", bufs=8) as apool, \
             tc.tile_pool(name="stage2", bufs=2) as spool, \
             tc.tile_pool(name="epi", bufs=2) as epool, \
             tc.tile_pool(name="spsum", bufs=2, space="PSUM") as pspool, \
             tc.tile_pool(name="opsum", bufs=2, space="PSUM") as opool, \
             tc.tile_pool(name="dram", bufs=1, space="DRAM") as dpool:

            idx1_t = cpool.tile([128, NCH], i32)
            nc.sync.dma_start(out=idx1_t[:], in_=t_idx1[:])
            idx2_t = cpool.tile([128, NCH], i32)
            nc.sync.dma_start(out=idx2_t[:], in_=t_idx2[:])
            dstc_t = cpool.tile([128, NCH], f32)
            nc.sync.dma_start(out=dstc_t[:], in_=t_dstc[:])
            sclc_t = cpool.tile([128, NCH], f32)
            nc.sync.dma_start(out=sclc_t[:], in_=t_sclc[:])
            x_myb = cpool.tile([128, NPP], f16)
            nc.sync.dma_start(out=x_myb[:], in_=t_xmyT[:])

            w_sb = cpool.tile([128, 2 * (T + 1) * F], f16)
            for k in range(2 * (T + 1)):
                nc.sync.dma_start(out=w_sb[:, k * F:(k + 1) * F], in_=t_wpack[k])
            b_sb = cpool.tile([1, 2 * F], f16)
            nc.sync.dma_start(out=b_sb[:, :F], in_=t_bpack[0:1, :])
            nc.sync.dma_start(out=b_sb[:, F:], in_=t_bpack[1:2, :])
            ones_sb = cpool.tile([1, 128], f16)
            nc.vector.memset(ones_sb[:], 1.0)
            eps_sb = cpool.tile([128, 1], f32)
            nc.vector.memset(eps_sb[:], 1e-24)

            iota_i = cpool.tile([128, 128], i32)
            nc.gpsimd.iota(iota_i[:], pattern=[[1, 128]], base=0, channel_multiplier=0)
            iota_f = cpool.tile([128, 128], f16)
            nc.vector.tensor_copy(out=iota_f[:], in_=iota_i[:])
            iotac_i = cpool.tile([128, 1], i32)
            nc.gpsimd.iota(iotac_i[:], pattern=[[1, 1]], base=0, channel_multiplier=1)
            iotac_f = cpool.tile([128, 1], f32)
            nc.vector.tensor_copy(out=iotac_f[:], in_=iotac_i[:])
            ident = cpool.tile([128, 128], f16)
            nc.vector.tensor_scalar(out=ident[:], in0=iota_f[:],
                                    scalar1=iotac_f[:], scalar2=None,
                                    op0=Alu.is_equal)

            h1buf = cpool.tile([128, NPP], f16)
            obuf0 = cpool.tile([128, NPP], f16)
            obuf1 = cpool.tile([128, NPP], f16)
            ss = cpool.tile([128, WPC], f32)
            nrm = cpool.tile([128, WPC], f32)
            rn = cpool.tile([128, WPC], f32)

            h1_my = dpool.tile([128, NPP], f16)
            h1_all = dpool.tile([C * NPP, F], f16, addr_space="Shared")

            batches = [(w0, min(w0 + GBW, WPC)) for w0 in range(0, WPC, GBW)]

            for layer in (0, 1):
                idx_t = idx1_t if layer == 0 else idx2_t
                src_tbl = t_x16 if layer == 0 else h1_all
                begin_a = 625 if layer == 0 else 784
                selfsrc = x_myb if layer == 0 else h1buf
                obuf = obuf0 if layer == 0 else obuf1
                wofs = layer * (T + 1) * F

                abuild_ctr = 0
                pend = None  # (w, s_sb, s_sbT) awaiting stage 2

                def stage2(pw, ps_sb, ps_sbT):
                    o_ps = opool.tile([128, 128], f32, space="PSUM", tag="o")
                    first = True
                    for t in range(T):
                        if prep["S_wt"][pw, t] == 0:
                            continue
                        nc.tensor.matmul(
                            out=o_ps[:], lhsT=ps_sb[:, t * F:(t + 1) * F],
                            rhs=w_sb[:, wofs + t * F:wofs + (t + 1) * F],
                            start=first, stop=False)
                        first = False
                    nc.tensor.matmul(
                        out=o_ps[:], lhsT=ps_sbT[:],
                        rhs=w_sb[:, wofs + T * F:wofs + (T + 1) * F],
                        start=first, stop=False)
                    nc.tensor.matmul(
                        out=o_ps[:], lhsT=ones_sb[:],
                        rhs=b_sb[:, layer * F:(layer + 1) * F],
                        start=False, stop=True)
                    nc.scalar.activation(out=obuf[:, pw * 128:(pw + 1) * 128],
                                         in_=o_ps[:], func=Act.Copy)
                    if layer == 0:
                        # row sum-of-squares straight from PSUM: avoids a DVE
                        # stall on the Act obuf copy
                        sq = epool.tile([128, 128], f16, tag="sq")
                        nc.vector.tensor_tensor_reduce(
                            out=sq[:], in0=o_ps[:], in1=o_ps[:],
                            scale=1.0, scalar=0.0, op0=Alu.mult, op1=Alu.add,
                            accum_out=ss[:, pw:pw + 1])

                for w0, w1 in batches:
                    ch0, ch1 = int(chbase[w0]), int(chbase[w1])
                    cw = ch1 - ch0
                    if cw > 0:
                        # 2-D reshape with a wide contiguous last dim: the
                        # cost model sizes descriptors from the innermost
                        # contiguous run, so per-row 256B descriptors (2x
                        # sub-512B latency penalty) become full-run ones.
                        # axis=1 makes the index coefficient 1 (indices are
                        # premultiplied by F on the host).
                        tbl2d = src_tbl[:].rearrange("(a c) b -> a (c b)", a=begin_a)
                        m_t = gpool.tile([128, cw * F], f16, tag="m")
                        nc.gpsimd.indirect_dma_start(
                            out=m_t[:], out_offset=None, in_=tbl2d,
                            in_offset=bass.IndirectOffsetOnAxis(
                                ap=idx_t[:, ch0:ch1], axis=1))
                    for w in range(w0, w1):
                        # stage 1: per-chunk scaled one-hot + matmul blocks
                        s_all = pspool.tile([128, T * 128], f32, space="PSUM",
                                            tag="sall")
                        s_self = pspool.tile([128, 128], f16, space="PSUM",
                                             tag="sself")
                        a_of_chunk = {}
                        for (ch, lo, hi, t, st, sp) in sched[w]:
                            if ch not in a_of_chunk:
                                a_t = apool.tile([128, 128], f16, tag="a")
                                eng = nc.gpsimd if abuild_ctr % 3 == 2 else nc.vector
                                abuild_ctr += 1
                                eng.tensor_scalar(
                                    out=a_t[:], in0=iota_f[:],
                                    scalar1=dstc_t[:, ch:ch + 1],
                                    scalar2=sclc_t[:, ch:ch + 1],
                                    op0=Alu.is_equal, op1=Alu.mult)
                                a_of_chunk[ch] = a_t
                            a_t = a_of_chunk[ch]
                            mc = (ch - ch0) * F
                            nc.tensor.matmul(
                                out=s_all[:, t * 128:(t + 1) * 128],
                                lhsT=m_t[lo:hi, mc:mc + F],
                                rhs=a_t[lo:hi, :], start=st, stop=sp)
                        nc.tensor.transpose(
                            out=s_self[:], in_=selfsrc[:, w * 128:(w + 1) * 128],
                            identity=ident[:])
                        # copies to SBUF for stage 2 (both on Act: DVE must
                        # stay free of PE-dependent ops or its in-order queue
                        # stalls the a-build pipeline)
                        s_sb = spool.tile([128, T * 128], f16, tag="ssb")
                        nc.scalar.activation(out=s_sb[:], in_=s_all[:],
                                             func=Act.Copy)
                        s_sbT = spool.tile([128, 128], f16, tag="ssbT")
                        nc.scalar.activation(out=s_sbT[:], in_=s_self[:],
                                             func=Act.Copy)
                        # stage 2 of the previous window (software pipeline)
                        if pend is not None:
                            stage2(*pend)
                        pend = (w, s_sb, s_sbT)
                if pend is not None:
                    stage2(*pend)

                if layer == 0:
                    nc.scalar.activation(out=nrm[:], in_=ss[:],
                                         func=Act.Sqrt, bias=eps_sb[:])
                    nc.vector.reciprocal(out=rn[:], in_=nrm[:])
                    for w in range(WPC):
                        nc.vector.tensor_scalar(
                            out=h1buf[:, w * 128:(w + 1) * 128],
                            in0=obuf0[:, w * 128:(w + 1) * 128],
                            scalar1=rn[:, w:w + 1], scalar2=0.0,
                            op0=Alu.mult, op1=Alu.max)
                    nc.sync.dma_start(out=h1_my[:], in_=h1buf[:])
                    nc.gpsimd.collective_compute(
                        "AllGather",
                        mybir.AluOpType.bypass,
                        replica_groups=[list(range(n_cores))],
                        ins=[h1_my[:]],
                        outs=[h1_all[:]],
                    )
                else:
                    nc.sync.dma_start(out=t_out[:], in_=obuf1[:])

    if legalize:
        _legalize_sync_waits(nc)
    return nc


def _unscramble(arr):
    # arr [128, NPP] fp16 partition-major -> [NPC, F] fp32 row-major
    return (arr.reshape(128, WPC, F).transpose(1, 0, 2)
            .reshape(NPP, F)[:NPC].astype(np.float32))


def kernel(**inputs):
    import sys
    if '/opt/trn_rl_repo' not in sys.path:
        sys.path.insert(0, '/opt/trn_rl_repo')

    prep = _prep(
        inputs["x"], inputs["W_self1"], inputs["W_neigh1"], inputs["b1"],
        inputs["W_self2"], inputs["W_neigh2"], inputs["b2"],
        inputs["edge_index"], inputs["edge_type"])

    nc = build_module(prep, legalize=True, n_cores=C)

    from concourse.bass_utils import run_bass_kernel_spmd
    in_maps = [
        {"x16": prep["x16"], "xmyT": prep["xmyT"][c], "idx1": prep["idx1"][c],
         "idx2": prep["idx2"][c], "dstc": prep["dstc"][c],
         "sclc": prep["sclc"][c], "wpack": prep["wpack"], "bpack": prep["bpack"]}
        for c in range(C)
    ]
    res = run_bass_kernel_spmd(nc, in_maps, core_ids=list(range(C)))

    out = np.empty((N, F), dtype=np.float32)
    for c in range(C):
        out[c * NPC:(c + 1) * NPC] = _unscramble(res.results[c]["out"])
    return out


# revision 16
# speedup vs baseline: 4.1177x; 1.0288x over previous
"""Trainium2 Bass kernel for nn_MetricalGNN (2-layer hetero GraphSAGE).

Math (per layer, T=4 edge types):
    out = h @ mean_t(W_self[t]) + mean_t(b[t])
        + (1/T) * sum_t diag(1/max(cnt_t,1)) @ segsum_t(h[src]) @ W_neigh[t]
Layer 1 is followed by row-wise L2 normalize + ReLU.

Device strategy (8 cores, destination-sharded, 6250 nodes each in 49
windows of 128 destinations):
  - Edges are sorted by (core, window, type).  Within a window each type
    segment is padded to a 64-slot boundary (slot count = max over cores,
    so all cores share one compile-time schedule); the window's slot list
    is padded to a 128 multiple and split into 128-edge chunks.
  - One batched indirect DMA per 7 windows gathers all source rows of
    those windows' chunks (128 rows per chunk) into SBUF as fp16.  This
    amortizes the ~1us SWDGE descriptor-generation cost per instruction
    that dominated the per-chunk-gather baseline.
  - Per chunk, one tensor_scalar builds the scaled one-hot
    A[e,d] = scale_e * (iota==dst_e) (4x DVE mode; every 3rd chunk runs
    on gpsimd instead to balance engines).  scale folds the 1/cnt mean,
    the 1/T type average, and padding (dst=300 -> zero column).
  - The TensorEngine accumulates S_t^T[f,d] += M^T A into per-type
    column slices of a single [128,512] PSUM tile; a chunk straddling a
    type boundary issues one matmul per 32-aligned partition sub-range
    (legal sub-ranges: start 0/32/64/96 within the quadrant tree).
  - The self term h_w @ W_self_avg needs h_w^T as stationary operand: a
    PE transpose of the SBUF-resident window tile (x / h1 slices stay in
    SBUF all layer, partition-major) produces it in PSUM fp16 - no DMA.
  - Stage 2 per window (software-pipelined one window behind stage 1):
    one Act copy moves the [128,512] S^T block to SBUF fp16, one DVE
    copy moves the self block, then 6 matmuls (4 neighbor + self + K=1
    ones-row bias) accumulate out[d,fo] in one PSUM bank.
  - Layer-1 epilogue fuses square+row-sum (tensor_tensor_reduce), then
    batched sqrt/reciprocal and a per-window scale+relu into the
    resident h1 buffer; one DMA stores h1 (partition-major) and an
    AllGather shares it; layer 2 gathers from the concatenated
    [8*6272,128] table via host-remapped indices.
"""

import numpy as np

N = 50000
E = 600000
F = 128
T = 4
C = 8                      # cores
NPC = N // C               # 6250 destinations per core
WPC = (NPC + 127) // 128   # 49 windows per core
NPP = WPC * 128            # 6272 padded rows per core slice
GBW = 7                    # windows per batched gather instruction
PAD_DST = 300.0            # one-hot miss -> zero column


def _qblocks(lo, hi):
    """Split [lo,hi) (64-aligned) into partition ranges legal for PE
    operands (base partition must be 0, 32, or 64)."""
    out = []
    while lo < hi:
        if lo == 0:
            out.append((0, hi)); lo = hi
        elif lo == 64:
            out.append((64, hi)); lo = hi
        else:
            raise AssertionError(lo)
    return out


def _prep(x, W_self1, W_neigh1, b1, W_self2, W_neigh2, b2, edge_index, edge_type):
    src = np.asarray(edge_index[0], dtype=np.int64)
    dst = np.asarray(edge_index[1], dtype=np.int64)
    et = np.asarray(edge_type, dtype=np.int64)

    cnt = np.bincount(et * N + dst, minlength=T * N).reshape(T, N).astype(np.float32)
    scale_e = (0.25 / np.maximum(cnt[et, dst], 1.0)).astype(np.float32)

    core = dst // NPC
    loc = dst % NPC
    win = loc // 128
    dloc = (loc % 128).astype(np.float32)

    # per (core, window, type) counts; shared slot allocation = max over cores
    key = (core * WPC + win) * T + et
    n_cwt = np.bincount(key, minlength=C * WPC * T).reshape(C, WPC, T)
    S_wt = 64 * ((n_cwt.max(axis=0) + 63) // 64)          # [WPC, T]
    Wslots = S_wt.sum(axis=1)                              # [WPC]
    Kw = (Wslots + 127) // 128                             # chunks per window
    chbase = np.zeros(WPC + 1, dtype=np.int64)
    np.cumsum(Kw, out=chbase[1:])
    NCH = int(chbase[-1])
    tbase = np.zeros((WPC, T), dtype=np.int64)             # slot base of type seg
    tbase[:, 1:] = np.cumsum(S_wt, axis=1)[:, :-1]

    # per-edge slot assignment (order within a (c,w,t) group is arbitrary)
    order = np.lexsort((et, win, core))
    key_s = key[order]
    grp_start_of_key = np.searchsorted(key_s, np.arange(C * WPC * T))
    within = np.arange(E) - grp_start_of_key[key_s]
    sbase_flat = (chbase[:WPC, None] * 128 + tbase).reshape(-1)  # [WPC*T]
    g = sbase_flat[(win * T + et)[order]] + within         # global slot per edge
    p_slot = (g % 128).astype(np.int64)
    ch_slot = (g // 128).astype(np.int64)

    idx1 = np.zeros((C, 128, NCH), dtype=np.int32)
    idx2 = np.zeros((C, 128, NCH), dtype=np.int32)
    dstc = np.full((C, 128, NCH), PAD_DST, dtype=np.float32)
    sclc = np.zeros((C, 128, NCH), dtype=np.float32)

    # indices premultiplied by F: gather tables are addressed as flat 1-D
    # arrays so each descriptor covers a whole per-partition run (>=512B,
    # avoiding the sub-512B descriptor latency penalty)
    src_s = src[order]
    csrc = src_s // NPC
    lsrc = src_s % NPC
    row2 = ((csrc * NPP + (lsrc % 128) * WPC + lsrc // 128) * F).astype(np.int32)
    core_s = core[order]
    idx1[core_s, p_slot, ch_slot] = (src_s * F).astype(np.int32)
    idx2[core_s, p_slot, ch_slot] = row2
    dstc[core_s, p_slot, ch_slot] = dloc[order]
    sclc[core_s, p_slot, ch_slot] = scale_e[order]

    # compile-time matmul schedule per window: (ch, lo, hi, t, start, stop)
    sched = []
    for w in range(WPC):
        blocks = []
        for t in range(T):
            if S_wt[w, t] == 0:
                continue
            s0, s1 = int(tbase[w, t]), int(tbase[w, t] + S_wt[w, t])
            tblocks = []
            for ch in range(int(chbase[w]), int(chbase[w + 1])):
                c0 = (ch - chbase[w]) * 128
                lo, hi = max(s0, c0), min(s1, c0 + 128)
                if lo < hi:
                    for ql, qh in _qblocks(lo - c0, hi - c0):
                        tblocks.append([ch, ql, qh, t, False, False])
            tblocks[0][4] = True
            tblocks[-1][5] = True
            blocks.extend(tuple(b) for b in tblocks)
        sched.append(blocks)

    # weights / bias packs
    wpack = np.empty((2 * (T + 1), F, F), dtype=np.float16)
    wpack[0:T] = np.asarray(W_neigh1, np.float32).astype(np.float16)
    wpack[T] = np.asarray(W_self1, np.float32).mean(axis=0).astype(np.float16)
    wpack[T + 1:2 * T + 1] = np.asarray(W_neigh2, np.float32).astype(np.float16)
    wpack[2 * T + 1] = np.asarray(W_self2, np.float32).mean(axis=0).astype(np.float16)
    bpack = np.stack([
        np.asarray(b1, np.float32).mean(axis=0),
        np.asarray(b2, np.float32).mean(axis=0),
    ]).astype(np.float16)

    x16 = np.asarray(x, np.float32).astype(np.float16)
    xpad = np.zeros((C, NPP, F), dtype=np.float16)
    for c in range(C):
        xpad[c, :NPC] = x16[c * NPC:(c + 1) * NPC]
    # partition-major my-slice: xmyT[c, p, w*128+f] = x[c*NPC + w*128 + p, f]
    xmyT = np.ascontiguousarray(
        xpad.reshape(C, WPC, 128, F).transpose(0, 2, 1, 3).reshape(C, 128, NPP))

    return dict(idx1=idx1, idx2=idx2, dstc=dstc, sclc=sclc, wpack=wpack,
                bpack=bpack, x16=x16, xmyT=xmyT, sched=sched, Kw=Kw,
                chbase=chbase, NCH=NCH, S_wt=S_wt)


def _legalize_sync_waits(nc, max_waits=1):
    """The walrus build in this container caps sync-wait commands per
    instruction; hoist excess waits onto NOPs inserted before the
    instruction on the same engine (sequencers execute in order)."""
    from concourse import mybir

    ctr = [0]
    for fn in nc.m.functions:
        for bb in fn.blocks:
            insts = bb.instructions
            if not any(
                i.sync_info is not None and len(i.sync_info.on_wait) > max_waits
                for i in insts
            ):
                continue
            out = []
            for inst in insts:
                si = inst.sync_info
                if si is not None and len(si.on_wait) > max_waits:
                    waits = list(si.on_wait)
                    keep = waits[-max_waits:]
                    hoist = waits[:-max_waits]
                    for i in range(0, len(hoist), max_waits):
                        nop = mybir.InstNoOp(
                            name=f"I-waitsplit-{ctr[0]}", ins=[], outs=[])
                        ctr[0] += 1
                        nop.engine = inst.engine
                        nop.sync_info = mybir.SyncInfo(
                            on_wait=hoist[i:i + max_waits], on_update=[])
                        out.append(nop)
                    inst.sync_info = mybir.SyncInfo(
                        on_wait=keep, on_update=list(si.on_update))
                out.append(inst)
            insts.clear()
            insts.extend(out)


def build_module(prep, legalize=True, n_cores=C):
    import concourse.bass as bass
    import concourse.tile as tile
    from concourse import mybir

    f16, f32, i32 = mybir.dt.float16, mybir.dt.float32, mybir.dt.int32
    Alu = mybir.AluOpType
    Act = mybir.ActivationFunctionType

    NCH = prep["NCH"]
    Kw = prep["Kw"]
    chbase = prep["chbase"]
    sched = prep["sched"]

    nc = bass.Bass(trn_type="TRN2")
    t_x16 = nc.dram_tensor("x16", [N, F], f16, kind="ExternalInput")
    t_xmyT = nc.dram_tensor("xmyT", [128, NPP], f16, kind="ExternalInput")
    t_idx1 = nc.dram_tensor("idx1", [128, NCH], i32, kind="ExternalInput")
    t_idx2 = nc.dram_tensor("idx2", [128, NCH], i32, kind="ExternalInput")
    t_dstc = nc.dram_tensor("dstc", [128, NCH], f32, kind="ExternalInput")
    t_sclc = nc.dram_tensor("sclc", [128, NCH], f32, kind="ExternalInput")
    t_wpack = nc.dram_tensor("wpack", [2 * (T + 1), F, F], f16, kind="ExternalInput")
    t_bpack = nc.dram_tensor("bpack", [2, F], f16, kind="ExternalInput")
    t_out = nc.dram_tensor("out", [128, NPP], f16, kind="ExternalOutput")

    with tile.TileContext(nc, num_cores=n_cores) as tc:
        with tc.tile_pool(name="const", bufs=1) as cpool, \
             tc.tile_pool(name="gath", bufs=2) as gpool, \
             tc.tile_pool(name="onehot", bufs=24) as apool, \
             tc.tile_pool(name="stage2", bufs=2) as spool, \
             tc.tile_pool(name="epi", bufs=2) as epool, \
             tc.tile_pool(name="spsum", bufs=2, space="PSUM") as pspool, \
             tc.tile_pool(name="opsum", bufs=2, space="PSUM") as opool, \
             tc.tile_pool(name="dram", bufs=1, space="DRAM") as dpool:

            idx1_t = cpool.tile([128, NCH], i32)
            nc.sync.dma_start(out=idx1_t[:], in_=t_idx1[:])
            idx2_t = cpool.tile([128, NCH], i32)
            nc.sync.dma_start(out=idx2_t[:], in_=t_idx2[:])
            dstc_t = cpool.tile([128, NCH], f32)
            nc.sync.dma_start(out=dstc_t[:], in_=t_dstc[:])
            sclc_t = cpool.tile([128, NCH], f32)
            nc.sync.dma_start(out=sclc_t[:], in_=t_sclc[:])
            x_myb = cpool.tile([128, NPP], f16)
            nc.sync.dma_start(out=x_myb[:], in_=t_xmyT[:])

            w_sb = cpool.tile([128, 2 * (T + 1) * F], f16)
            for k in range(2 * (T + 1)):
                nc.sync.dma_start(out=w_sb[:, k * F:(k + 1) * F], in_=t_wpack[k])
            b_sb = cpool.tile([1, 2 * F], f16)
            nc.sync.dma_start(out=b_sb[:, :F], in_=t_bpack[0:1, :])
            nc.sync.dma_start(out=b_sb[:, F:], in_=t_bpack[1:2, :])
            ones_sb = cpool.tile([1, 128], f16)
            nc.vector.memset(ones_sb[:], 1.0)
            eps_sb = cpool.tile([128, 1], f32)
            nc.vector.memset(eps_sb[:], 1e-24)

            iota_i = cpool.tile([128, 128], i32)
            nc.gpsimd.iota(iota_i[:], pattern=[[1, 128]], base=0, channel_multiplier=0)
            iota_f = cpool.tile([128, 128], f16)
            nc.vector.tensor_copy(out=iota_f[:], in_=iota_i[:])
            iotac_i = cpool.tile([128, 1], i32)
            nc.gpsimd.iota(iotac_i[:], pattern=[[1, 1]], base=0, channel_multiplier=1)
            iotac_f = cpool.tile([128, 1], f32)
            nc.vector.tensor_copy(out=iotac_f[:], in_=iotac_i[:])
            ident = cpool.tile([128, 128], f16)
            nc.vector.tensor_scalar(out=ident[:], in0=iota_f[:],
                                    scalar1=iotac_f[:], scalar2=None,
                                    op0=Alu.is_equal)

            h1buf = cpool.tile([128, NPP], f16)
            obuf0 = cpool.tile([128, NPP], f16)
            obuf1 = cpool.tile([128, NPP], f16)
            ss = cpool.tile([128, WPC], f32)
            nrm = cpool.tile([128, WPC], f32)
            rn = cpool.tile([128, WPC], f32)

            h1_my = dpool.tile([128, NPP], f16)
            h1_all = dpool.tile([C * NPP, F], f16, addr_space="Shared")

            batches = [(w0, min(w0 + GBW, WPC)) for w0 in range(0, WPC, GBW)]

            for layer in (0, 1):
                idx_t = idx1_t if layer == 0 else idx2_t
                src_tbl = t_x16 if layer == 0 else h1_all
                begin_a = 625 if layer == 0 else 784
                selfsrc = x_myb if layer == 0 else h1buf
                obuf = obuf0 if layer == 0 else obuf1
                wofs = layer * (T + 1) * F

                abuild_ctr = 0
                pend = None  # (w, s_sb, s_sbT) awaiting stage 2
                ttr_queue = []  # layer-0 windows awaiting sum-of-squares
                TTR_LAG = 8  # keep DVE several windows behind PE/Act

                def emit_ttr(pw):
                    # row sum-of-squares from the resident obuf slice; lagged
                    # so the DVE never waits on Act's copy
                    sq = epool.tile([128, 128], f16, tag="sq")
                    nc.vector.tensor_tensor_reduce(
                        out=sq[:], in0=obuf[:, pw * 128:(pw + 1) * 128],
                        in1=obuf[:, pw * 128:(pw + 1) * 128],
                        scale=1.0, scalar=0.0, op0=Alu.mult, op1=Alu.add,
                        accum_out=ss[:, pw:pw + 1])

                def stage2(pw, ps_sb, ps_sbT):
                    o_ps = opool.tile([128, 128], f32, space="PSUM", tag="o")
                    first = True
                    for t in range(T):
                        if prep["S_wt"][pw, t] == 0:
                            continue
                        nc.tensor.matmul(
                            out=o_ps[:], lhsT=ps_sb[:, t * F:(t + 1) * F],
                            rhs=w_sb[:, wofs + t * F:wofs + (t + 1) * F],
                            start=first, stop=False)
                        first = False
                    nc.tensor.matmul(
                        out=o_ps[:], lhsT=ps_sbT[:],
                        rhs=w_sb[:, wofs + T * F:wofs + (T + 1) * F],
                        start=first, stop=False)
                    nc.tensor.matmul(
                        out=o_ps[:], lhsT=ones_sb[:],
                        rhs=b_sb[:, layer * F:(layer + 1) * F],
                        start=False, stop=True)
                    nc.scalar.activation(out=obuf[:, pw * 128:(pw + 1) * 128],
                                         in_=o_ps[:], func=Act.Copy)

                for w0, w1 in batches:
                    ch0, ch1 = int(chbase[w0]), int(chbase[w1])
                    cw = ch1 - ch0
                    if cw > 0:
                        # 2-D reshape with a wide contiguous last dim: the
                        # cost model sizes descriptors from the innermost
                        # contiguous run, so per-row 256B descriptors (2x
                        # sub-512B latency penalty) become full-run ones.
                        # axis=1 makes the index coefficient 1 (indices are
                        # premultiplied by F on the host).
                        tbl2d = src_tbl[:].rearrange("(a c) b -> a (c b)", a=begin_a)
                        m_t = gpool.tile([128, cw * F], f16, tag="m")
                        nc.gpsimd.indirect_dma_start(
                            out=m_t[:], out_offset=None, in_=tbl2d,
                            in_offset=bass.IndirectOffsetOnAxis(
                                ap=idx_t[:, ch0:ch1], axis=1))
                    for w in range(w0, w1):
                        # stage 1: per-chunk scaled one-hot + matmul blocks
                        s_all = pspool.tile([128, T * 128], f32, space="PSUM",
                                            tag="sall")
                        s_self = pspool.tile([128, 128], f16, space="PSUM",
                                             tag="sself")
                        a_of_chunk = {}
                        for (ch, lo, hi, t, st, sp) in sched[w]:
                            if ch not in a_of_chunk:
                                a_t = apool.tile([128, 128], f16, tag="a")
                                eng = nc.gpsimd if abuild_ctr % 3 == 2 else nc.vector
                                abuild_ctr += 1
                                eng.tensor_scalar(
                                    out=a_t[:], in0=iota_f[:],
                                    scalar1=dstc_t[:, ch:ch + 1],
                                    scalar2=sclc_t[:, ch:ch + 1],
                                    op0=Alu.is_equal, op1=Alu.mult)
                                a_of_chunk[ch] = a_t
                            a_t = a_of_chunk[ch]
                            mc = (ch - ch0) * F
                            nc.tensor.matmul(
                                out=s_all[:, t * 128:(t + 1) * 128],
                                lhsT=m_t[lo:hi, mc:mc + F],
                                rhs=a_t[lo:hi, :], start=st, stop=sp)
                        nc.tensor.transpose(
                            out=s_self[:], in_=selfsrc[:, w * 128:(w + 1) * 128],
                            identity=ident[:])
                        # copies to SBUF for stage 2 (both on Act: DVE must
                        # stay free of PE-dependent ops or its in-order queue
                        # stalls the a-build pipeline)
                        s_sb = spool.tile([128, T * 128], f16, tag="ssb")
                        nc.scalar.activation(out=s_sb[:], in_=s_all[:],
                                             func=Act.Copy)
                        s_sbT = spool.tile([128, 128], f16, tag="ssbT")
                        nc.scalar.activation(out=s_sbT[:], in_=s_self[:],
                                             func=Act.Copy)
                        # stage 2 of the previous window (software pipeline)
                        if pend is not None:
                            stage2(*pend)
                            if layer == 0:
                                ttr_queue.append(pend[0])
                                if len(ttr_queue) > TTR_LAG:
                                    emit_ttr(ttr_queue.pop(0))
                        pend = (w, s_sb, s_sbT)
                if pend is not None:
                    stage2(*pend)
                    if layer == 0:
                        ttr_queue.append(pend[0])
                for pw in ttr_queue:
                    emit_ttr(pw)

                if layer == 0:
                    nc.scalar.activation(out=nrm[:], in_=ss[:],
                                         func=Act.Sqrt, bias=eps_sb[:])
                    nc.vector.reciprocal(out=rn[:], in_=nrm[:])
                    for w in range(WPC):
                        nc.vector.tensor_scalar(
                            out=h1buf[:, w * 128:(w + 1) * 128],
                            in0=obuf0[:, w * 128:(w + 1) * 128],
                            scalar1=rn[:, w:w + 1], scalar2=0.0,
                            op0=Alu.mult, op1=Alu.max)
                    nc.sync.dma_start(out=h1_my[:], in_=h1buf[:])
                    nc.gpsimd.collective_compute(
                        "AllGather",
                        mybir.AluOpType.bypass,
                        replica_groups=[list(range(n_cores))],
                        ins=[h1_my[:]],
                        outs=[h1_all[:]],
                    )
                else:
                    nc.sync.dma_start(out=t_out[:], in_=obuf1[:])

    if legalize:
        _legalize_sync_waits(nc)
    return nc


def _unscramble(arr):
    # arr [128, NPP] fp16 partition-major -> [NPC, F] fp32 row-major
    return (arr.reshape(128, WPC, F).transpose(1, 0, 2)
            .reshape(NPP, F)[:NPC].astype(np.float32))


def kernel(**inputs):
    import sys
    if '/opt/trn_rl_repo' not in sys.path:
        sys.path.insert(0, '/opt/trn_rl_repo')

    prep = _prep(
        inputs["x"], inputs["W_self1"], inputs["W_neigh1"], inputs["b1"],
        inputs["W_self2"], inputs["W_neigh2"], inputs["b2"],
        inputs["edge_index"], inputs["edge_type"])

    nc = build_module(prep, legalize=True, n_cores=C)

    from concourse.bass_utils import run_bass_kernel_spmd
    in_maps = [
        {"x16": prep["x16"], "xmyT": prep["xmyT"][c], "idx1": prep["idx1"][c],
         "idx2": prep["idx2"][c], "dstc": prep["dstc"][c],
         "sclc": prep["sclc"][c], "wpack": prep["wpack"], "bpack": prep["bpack"]}
        for c in range(C)
    ]
    res = run_bass_kernel_spmd(nc, in_maps, core_ids=list(range(C)))

    out = np.empty((N, F), dtype=np.float32)
    for c in range(C):
        out[c * NPC:(c + 1) * NPC] = _unscramble(res.results[c]["out"])
    return out


# revision 18
# speedup vs baseline: 4.1634x; 1.0111x over previous
"""Trainium2 Bass kernel for nn_MetricalGNN (2-layer hetero GraphSAGE).

Math (per layer, T=4 edge types):
    out = h @ mean_t(W_self[t]) + mean_t(b[t])
        + (1/T) * sum_t diag(1/max(cnt_t,1)) @ segsum_t(h[src]) @ W_neigh[t]
Layer 1 is followed by row-wise L2 normalize + ReLU.

Device strategy (8 cores, destination-sharded, 6250 nodes each in 49
windows of 128 destinations):
  - Edges are sorted by (core, window, type).  Within a window each type
    segment is padded to a 64-slot boundary (slot count = max over cores,
    so all cores share one compile-time schedule); the window's slot list
    is padded to a 128 multiple and split into 128-edge chunks.
  - One batched indirect DMA per 7 windows gathers all source rows of
    those windows' chunks (128 rows per chunk) into SBUF as fp16.  This
    amortizes the ~1us SWDGE descriptor-generation cost per instruction
    that dominated the per-chunk-gather baseline.
  - Per chunk, one tensor_scalar builds the scaled one-hot
    A[e,d] = scale_e * (iota==dst_e) (4x DVE mode; every 3rd chunk runs
    on gpsimd instead to balance engines).  scale folds the 1/cnt mean,
    the 1/T type average, and padding (dst=300 -> zero column).
  - The TensorEngine accumulates S_t^T[f,d] += M^T A into per-type
    column slices of a single [128,512] PSUM tile; a chunk straddling a
    type boundary issues one matmul per 32-aligned partition sub-range
    (legal sub-ranges: start 0/32/64/96 within the quadrant tree).
  - The self term h_w @ W_self_avg needs h_w^T as stationary operand: a
    PE transpose of the SBUF-resident window tile (x / h1 slices stay in
    SBUF all layer, partition-major) produces it in PSUM fp16 - no DMA.
  - Stage 2 per window (software-pipelined one window behind stage 1):
    one Act copy moves the [128,512] S^T block to SBUF fp16, one DVE
    copy moves the self block, then 6 matmuls (4 neighbor + self + K=1
    ones-row bias) accumulate out[d,fo] in one PSUM bank.
  - Layer-1 epilogue fuses square+row-sum (tensor_tensor_reduce), then
    batched sqrt/reciprocal and a per-window scale+relu into the
    resident h1 buffer; one DMA stores h1 (partition-major) and an
    AllGather shares it; layer 2 gathers from the concatenated
    [8*6272,128] table via host-remapped indices.
"""

import numpy as np

N = 50000
E = 600000
F = 128
T = 4
C = 8                      # cores
NPC = N // C               # 6250 destinations per core
WPC = (NPC + 127) // 128   # 49 windows per core
NPP = WPC * 128            # 6272 padded rows per core slice
GBW = 5                    # windows per batched gather instruction
PAD_DST = 300.0            # one-hot miss -> zero column


def _qblocks(lo, hi):
    """Split [lo,hi) (64-aligned) into partition ranges legal for PE
    operands (base partition must be 0, 32, or 64)."""
    out = []
    while lo < hi:
        if lo == 0:
            out.append((0, hi)); lo = hi
        elif lo == 64:
            out.append((64, hi)); lo = hi
        else:
            raise AssertionError(lo)
    return out


def _prep(x, W_self1, W_neigh1, b1, W_self2, W_neigh2, b2, edge_index, edge_type):
    src = np.asarray(edge_index[0], dtype=np.int64)
    dst = np.asarray(edge_index[1], dtype=np.int64)
    et = np.asarray(edge_type, dtype=np.int64)

    cnt = np.bincount(et * N + dst, minlength=T * N).reshape(T, N).astype(np.float32)
    scale_e = (0.25 / np.maximum(cnt[et, dst], 1.0)).astype(np.float32)

    core = dst // NPC
    loc = dst % NPC
    win = loc // 128
    dloc = (loc % 128).astype(np.float32)

    # per (core, window, type) counts; shared slot allocation = max over cores
    key = (core * WPC + win) * T + et
    n_cwt = np.bincount(key, minlength=C * WPC * T).reshape(C, WPC, T)
    S_wt = 64 * ((n_cwt.max(axis=0) + 63) // 64)          # [WPC, T]
    Wslots = S_wt.sum(axis=1)                              # [WPC]
    Kw = (Wslots + 127) // 128                             # chunks per window
    chbase = np.zeros(WPC + 1, dtype=np.int64)
    np.cumsum(Kw, out=chbase[1:])
    NCH = int(chbase[-1])
    tbase = np.zeros((WPC, T), dtype=np.int64)             # slot base of type seg
    tbase[:, 1:] = np.cumsum(S_wt, axis=1)[:, :-1]

    # per-edge slot assignment (order within a (c,w,t) group is arbitrary)
    order = np.lexsort((et, win, core))
    key_s = key[order]
    grp_start_of_key = np.searchsorted(key_s, np.arange(C * WPC * T))
    within = np.arange(E) - grp_start_of_key[key_s]
    sbase_flat = (chbase[:WPC, None] * 128 + tbase).reshape(-1)  # [WPC*T]
    g = sbase_flat[(win * T + et)[order]] + within         # global slot per edge
    p_slot = (g % 128).astype(np.int64)
    ch_slot = (g // 128).astype(np.int64)

    idx1 = np.zeros((C, 128, NCH), dtype=np.int32)
    idx2 = np.zeros((C, 128, NCH), dtype=np.int32)
    dstc = np.full((C, 128, NCH), PAD_DST, dtype=np.float32)
    sclc = np.zeros((C, 128, NCH), dtype=np.float32)

    # indices premultiplied by F: gather tables are addressed as flat 1-D
    # arrays so each descriptor covers a whole per-partition run (>=512B,
    # avoiding the sub-512B descriptor latency penalty)
    src_s = src[order]
    csrc = src_s // NPC
    lsrc = src_s % NPC
    row2 = ((csrc * NPP + (lsrc % 128) * WPC + lsrc // 128) * F).astype(np.int32)
    core_s = core[order]
    idx1[core_s, p_slot, ch_slot] = (src_s * F).astype(np.int32)
    idx2[core_s, p_slot, ch_slot] = row2
    dstc[core_s, p_slot, ch_slot] = dloc[order]
    sclc[core_s, p_slot, ch_slot] = scale_e[order]

    # compile-time matmul schedule per window: (ch, lo, hi, t, start, stop)
    sched = []
    for w in range(WPC):
        blocks = []
        for t in range(T):
            if S_wt[w, t] == 0:
                continue
            s0, s1 = int(tbase[w, t]), int(tbase[w, t] + S_wt[w, t])
            tblocks = []
            for ch in range(int(chbase[w]), int(chbase[w + 1])):
                c0 = (ch - chbase[w]) * 128
                lo, hi = max(s0, c0), min(s1, c0 + 128)
                if lo < hi:
                    for ql, qh in _qblocks(lo - c0, hi - c0):
                        tblocks.append([ch, ql, qh, t, False, False])
            tblocks[0][4] = True
            tblocks[-1][5] = True
            blocks.extend(tuple(b) for b in tblocks)
        sched.append(blocks)

    # weights / bias packs
    wpack = np.empty((2 * (T + 1), F, F), dtype=np.float16)
    wpack[0:T] = np.asarray(W_neigh1, np.float32).astype(np.float16)
    wpack[T] = np.asarray(W_self1, np.float32).mean(axis=0).astype(np.float16)
    wpack[T + 1:2 * T + 1] = np.asarray(W_neigh2, np.float32).astype(np.float16)
    wpack[2 * T + 1] = np.asarray(W_self2, np.float32).mean(axis=0).astype(np.float16)
    bpack = np.stack([
        np.asarray(b1, np.float32).mean(axis=0),
        np.asarray(b2, np.float32).mean(axis=0),
    ]).astype(np.float16)

    x16 = np.asarray(x, np.float32).astype(np.float16)
    xpad = np.zeros((C, NPP, F), dtype=np.float16)
    for c in range(C):
        xpad[c, :NPC] = x16[c * NPC:(c + 1) * NPC]
    # partition-major my-slice: xmyT[c, p, w*128+f] = x[c*NPC + w*128 + p, f]
    xmyT = np.ascontiguousarray(
        xpad.reshape(C, WPC, 128, F).transpose(0, 2, 1, 3).reshape(C, 128, NPP))

    return dict(idx1=idx1, idx2=idx2, dstc=dstc, sclc=sclc, wpack=wpack,
                bpack=bpack, x16=x16, xmyT=xmyT, sched=sched, Kw=Kw,
                chbase=chbase, NCH=NCH, S_wt=S_wt)


def _legalize_sync_waits(nc, max_waits=1):
    """The walrus build in this container caps sync-wait commands per
    instruction; hoist excess waits onto NOPs inserted before the
    instruction on the same engine (sequencers execute in order)."""
    from concourse import mybir

    ctr = [0]
    for fn in nc.m.functions:
        for bb in fn.blocks:
            insts = bb.instructions
            if not any(
                i.sync_info is not None and len(i.sync_info.on_wait) > max_waits
                for i in insts
            ):
                continue
            out = []
            for inst in insts:
                si = inst.sync_info
                if si is not None and len(si.on_wait) > max_waits:
                    waits = list(si.on_wait)
                    keep = waits[-max_waits:]
                    hoist = waits[:-max_waits]
                    for i in range(0, len(hoist), max_waits):
                        nop = mybir.InstNoOp(
                            name=f"I-waitsplit-{ctr[0]}", ins=[], outs=[])
                        ctr[0] += 1
                        nop.engine = inst.engine
                        nop.sync_info = mybir.SyncInfo(
                            on_wait=hoist[i:i + max_waits], on_update=[])
                        out.append(nop)
                    inst.sync_info = mybir.SyncInfo(
                        on_wait=keep, on_update=list(si.on_update))
                out.append(inst)
            insts.clear()
            insts.extend(out)


def build_module(prep, legalize=True, n_cores=C):
    import concourse.bass as bass
    import concourse.tile as tile
    from concourse import mybir

    f16, f32, i32 = mybir.dt.float16, mybir.dt.float32, mybir.dt.int32
    Alu = mybir.AluOpType
    Act = mybir.ActivationFunctionType

    NCH = prep["NCH"]
    Kw = prep["Kw"]
    chbase = prep["chbase"]
    sched = prep["sched"]

    nc = bass.Bass(trn_type="TRN2")
    t_x16 = nc.dram_tensor("x16", [N, F], f16, kind="ExternalInput")
    t_xmyT = nc.dram_tensor("xmyT", [128, NPP], f16, kind="ExternalInput")
    t_idx1 = nc.dram_tensor("idx1", [128, NCH], i32, kind="ExternalInput")
    t_idx2 = nc.dram_tensor("idx2", [128, NCH], i32, kind="ExternalInput")
    t_dstc = nc.dram_tensor("dstc", [128, NCH], f32, kind="ExternalInput")
    t_sclc = nc.dram_tensor("sclc", [128, NCH], f32, kind="ExternalInput")
    t_wpack = nc.dram_tensor("wpack", [2 * (T + 1), F, F], f16, kind="ExternalInput")
    t_bpack = nc.dram_tensor("bpack", [2, F], f16, kind="ExternalInput")
    t_out = nc.dram_tensor("out", [128, NPP], f16, kind="ExternalOutput")

    with tile.TileContext(nc, num_cores=n_cores) as tc:
        with tc.tile_pool(name="const", bufs=1) as cpool, \
             tc.tile_pool(name="gath", bufs=3) as gpool, \
             tc.tile_pool(name="onehot", bufs=24) as apool, \
             tc.tile_pool(name="stage2", bufs=2) as spool, \
             tc.tile_pool(name="epi", bufs=2) as epool, \
             tc.tile_pool(name="spsum", bufs=2, space="PSUM") as pspool, \
             tc.tile_pool(name="opsum", bufs=2, space="PSUM") as opool, \
             tc.tile_pool(name="dram", bufs=1, space="DRAM") as dpool:

            idx1_t = cpool.tile([128, NCH], i32)
            nc.sync.dma_start(out=idx1_t[:], in_=t_idx1[:])
            idx2_t = cpool.tile([128, NCH], i32)
            nc.sync.dma_start(out=idx2_t[:], in_=t_idx2[:])
            dstc_t = cpool.tile([128, NCH], f32)
            nc.sync.dma_start(out=dstc_t[:], in_=t_dstc[:])
            sclc_t = cpool.tile([128, NCH], f32)
            nc.sync.dma_start(out=sclc_t[:], in_=t_sclc[:])
            x_myb = cpool.tile([128, NPP], f16)
            nc.sync.dma_start(out=x_myb[:], in_=t_xmyT[:])

            w_sb = cpool.tile([128, 2 * (T + 1) * F], f16)
            for k in range(2 * (T + 1)):
                nc.sync.dma_start(out=w_sb[:, k * F:(k + 1) * F], in_=t_wpack[k])
            b_sb = cpool.tile([1, 2 * F], f16)
            nc.sync.dma_start(out=b_sb[:, :F], in_=t_bpack[0:1, :])
            nc.sync.dma_start(out=b_sb[:, F:], in_=t_bpack[1:2, :])
            ones_sb = cpool.tile([1, 128], f16)
            nc.vector.memset(ones_sb[:], 1.0)
            eps_sb = cpool.tile([128, 1], f32)
            nc.vector.memset(eps_sb[:], 1e-24)

            iota_i = cpool.tile([128, 128], i32)
            nc.gpsimd.iota(iota_i[:], pattern=[[1, 128]], base=0, channel_multiplier=0)
            iota_f = cpool.tile([128, 128], f16)
            nc.vector.tensor_copy(out=iota_f[:], in_=iota_i[:])
            iotac_i = cpool.tile([128, 1], i32)
            nc.gpsimd.iota(iotac_i[:], pattern=[[1, 1]], base=0, channel_multiplier=1)
            iotac_f = cpool.tile([128, 1], f32)
            nc.vector.tensor_copy(out=iotac_f[:], in_=iotac_i[:])
            ident = cpool.tile([128, 128], f16)
            nc.vector.tensor_scalar(out=ident[:], in0=iota_f[:],
                                    scalar1=iotac_f[:], scalar2=None,
                                    op0=Alu.is_equal)

            h1buf = cpool.tile([128, NPP], f16)
            obuf0 = cpool.tile([128, NPP], f16)
            obuf1 = cpool.tile([128, NPP], f16)
            ss = cpool.tile([128, WPC], f32)
            nrm = cpool.tile([128, WPC], f32)
            rn = cpool.tile([128, WPC], f32)

            h1_my = dpool.tile([128, NPP], f16)
            h1_all = dpool.tile([C * NPP, F], f16, addr_space="Shared")

            batches = [(w0, min(w0 + GBW, WPC)) for w0 in range(0, WPC, GBW)]

            for layer in (0, 1):
                idx_t = idx1_t if layer == 0 else idx2_t
                src_tbl = t_x16 if layer == 0 else h1_all
                begin_a = 625 if layer == 0 else 784
                selfsrc = x_myb if layer == 0 else h1buf
                obuf = obuf0 if layer == 0 else obuf1
                wofs = layer * (T + 1) * F

                abuild_ctr = 0
                pend = None  # (w, s_sb, s_sbT) awaiting stage 2
                ttr_queue = []  # layer-0 windows awaiting sum-of-squares
                TTR_LAG = 8  # keep DVE several windows behind PE/Act

                def emit_ttr(pw):
                    # row sum-of-squares from the resident obuf slice; lagged
                    # so the DVE never waits on Act's copy
                    sq = epool.tile([128, 128], f16, tag="sq")
                    nc.vector.tensor_tensor_reduce(
                        out=sq[:], in0=obuf[:, pw * 128:(pw + 1) * 128],
                        in1=obuf[:, pw * 128:(pw + 1) * 128],
                        scale=1.0, scalar=0.0, op0=Alu.mult, op1=Alu.add,
                        accum_out=ss[:, pw:pw + 1])

                def stage2(pw, ps_sb, ps_sbT):
                    o_ps = opool.tile([128, 128], f32, space="PSUM", tag="o")
                    first = True
                    for t in range(T):
                        if prep["S_wt"][pw, t] == 0:
                            continue
                        nc.tensor.matmul(
                            out=o_ps[:], lhsT=ps_sb[:, t * F:(t + 1) * F],
                            rhs=w_sb[:, wofs + t * F:wofs + (t + 1) * F],
                            start=first, stop=False)
                        first = False
                    nc.tensor.matmul(
                        out=o_ps[:], lhsT=ps_sbT[:],
                        rhs=w_sb[:, wofs + T * F:wofs + (T + 1) * F],
                        start=first, stop=False)
                    nc.tensor.matmul(
                        out=o_ps[:], lhsT=ones_sb[:],
                        rhs=b_sb[:, layer * F:(layer + 1) * F],
                        start=False, stop=True)
                    nc.scalar.activation(out=obuf[:, pw * 128:(pw + 1) * 128],
                                         in_=o_ps[:], func=Act.Copy)

                def emit_gather(bi):
                    # 2-D reshape with a wide contiguous last dim: the cost
                    # model sizes descriptors from the innermost contiguous
                    # run, so per-row 256B descriptors (2x sub-512B latency
                    # penalty) become full-run ones.  axis=1 makes the index
                    # coefficient 1 (indices are premultiplied by F on the
                    # host).
                    gw0, gw1 = batches[bi]
                    gc0, gc1 = int(chbase[gw0]), int(chbase[gw1])
                    if gc1 == gc0:
                        return None
                    tbl2d = src_tbl[:].rearrange("(a c) b -> a (c b)", a=begin_a)
                    m_t = gpool.tile([128, (gc1 - gc0) * F], f16, tag="m")
                    nc.gpsimd.indirect_dma_start(
                        out=m_t[:], out_offset=None, in_=tbl2d,
                        in_offset=bass.IndirectOffsetOnAxis(
                            ap=idx_t[:, gc0:gc1], axis=1))
                    return m_t

                # prefetch gathers two batches ahead (m pool bufs=3) so the
                # indirect DMA of batch b+2 overlaps batch b's compute
                mtiles = {}
                for bi in range(min(2, len(batches))):
                    mtiles[bi] = emit_gather(bi)

                for bi, (w0, w1) in enumerate(batches):
                    if bi + 2 < len(batches):
                        mtiles[bi + 2] = emit_gather(bi + 2)
                    ch0 = int(chbase[w0])
                    m_t = mtiles.pop(bi)
                    for w in range(w0, w1):
                        # stage 1: per-chunk scaled one-hot + matmul blocks
                        s_all = pspool.tile([128, T * 128], f32, space="PSUM",
                                            tag="sall")
                        s_self = pspool.tile([128, 128], f16, space="PSUM",
                                             tag="sself")
                        a_of_chunk = {}
                        for (ch, lo, hi, t, st, sp) in sched[w]:
                            if ch not in a_of_chunk:
                                a_t = apool.tile([128, 128], f16, tag="a")
                                eng = nc.gpsimd if abuild_ctr % 3 == 2 else nc.vector
                                abuild_ctr += 1
                                eng.tensor_scalar(
                                    out=a_t[:], in0=iota_f[:],
                                    scalar1=dstc_t[:, ch:ch + 1],
                                    scalar2=sclc_t[:, ch:ch + 1],
                                    op0=Alu.is_equal, op1=Alu.mult)
                                a_of_chunk[ch] = a_t
                            a_t = a_of_chunk[ch]
                            mc = (ch - ch0) * F
                            nc.tensor.matmul(
                                out=s_all[:, t * 128:(t + 1) * 128],
                                lhsT=m_t[lo:hi, mc:mc + F],
                                rhs=a_t[lo:hi, :], start=st, stop=sp)
                        nc.tensor.transpose(
                            out=s_self[:], in_=selfsrc[:, w * 128:(w + 1) * 128],
                            identity=ident[:])
                        # copies to SBUF for stage 2 (both on Act: DVE must
                        # stay free of PE-dependent ops or its in-order queue
                        # stalls the a-build pipeline)
                        s_sb = spool.tile([128, T * 128], f16, tag="ssb")
                        nc.scalar.activation(out=s_sb[:], in_=s_all[:],
                                             func=Act.Copy)
                        s_sbT = spool.tile([128, 128], f16, tag="ssbT")
                        nc.scalar.activation(out=s_sbT[:], in_=s_self[:],
                                             func=Act.Copy)
                        # stage 2 of the previous window (software pipeline)
                        if pend is not None:
                            stage2(*pend)
                            if layer == 0:
                                ttr_queue.append(pend[0])
                                if len(ttr_queue) > TTR_LAG:
                                    emit_ttr(ttr_queue.pop(0))
                        pend = (w, s_sb, s_sbT)
                if pend is not None:
                    stage2(*pend)
                    if layer == 0:
                        ttr_queue.append(pend[0])
                for pw in ttr_queue:
                    emit_ttr(pw)

                if layer == 0:
                    nc.scalar.activation(out=nrm[:], in_=ss[:],
                                         func=Act.Sqrt, bias=eps_sb[:])
                    nc.vector.reciprocal(out=rn[:], in_=nrm[:])
                    for w in range(WPC):
                        nc.vector.tensor_scalar(
                            out=h1buf[:, w * 128:(w + 1) * 128],
                            in0=obuf0[:, w * 128:(w + 1) * 128],
                            scalar1=rn[:, w:w + 1], scalar2=0.0,
                            op0=Alu.mult, op1=Alu.max)
                    nc.sync.dma_start(out=h1_my[:], in_=h1buf[:])
                    nc.gpsimd.collective_compute(
                        "AllGather",
                        mybir.AluOpType.bypass,
                        replica_groups=[list(range(n_cores))],
                        ins=[h1_my[:]],
                        outs=[h1_all[:]],
                    )
                else:
                    nc.sync.dma_start(out=t_out[:], in_=obuf1[:])

    if legalize:
        _legalize_sync_waits(nc)
    return nc


def _unscramble(arr):
    # arr [128, NPP] fp16 partition-major -> [NPC, F] fp32 row-major
    return (arr.reshape(128, WPC, F).transpose(1, 0, 2)
            .reshape(NPP, F)[:NPC].astype(np.float32))


def kernel(**inputs):
    import sys
    if '/opt/trn_rl_repo' not in sys.path:
        sys.path.insert(0, '/opt/trn_rl_repo')

    prep = _prep(
        inputs["x"], inputs["W_self1"], inputs["W_neigh1"], inputs["b1"],
        inputs["W_self2"], inputs["W_neigh2"], inputs["b2"],
        inputs["edge_index"], inputs["edge_type"])

    nc = build_module(prep, legalize=True, n_cores=C)

    from concourse.bass_utils import run_bass_kernel_spmd
    in_maps = [
        {"x16": prep["x16"], "xmyT": prep["xmyT"][c], "idx1": prep["idx1"][c],
         "idx2": prep["idx2"][c], "dstc": prep["dstc"][c],
         "sclc": prep["sclc"][c], "wpack": prep["wpack"], "bpack": prep["bpack"]}
        for c in range(C)
    ]
    res = run_bass_kernel_spmd(nc, in_maps, core_ids=list(range(C)))

    out = np.empty((N, F), dtype=np.float32)
    for c in range(C):
        out[c * NPC:(c + 1) * NPC] = _unscramble(res.results[c]["out"])
    return out


# revision 22
# speedup vs baseline: 4.5244x; 1.0867x over previous
"""Trainium2 Bass kernel for nn_MetricalGNN (2-layer hetero GraphSAGE).

Math (per layer, T=4 edge types):
    out = h @ mean_t(W_self[t]) + mean_t(b[t])
        + (1/T) * sum_t diag(1/max(cnt_t,1)) @ segsum_t(h[src]) @ W_neigh[t]
Layer 1 is followed by row-wise L2 normalize + ReLU.

Device strategy (8 cores, destination-sharded, 6250 nodes each in 49
windows of 128 destinations):
  - Edges are sorted by (core, window, type).  Within a window each type
    segment is padded to a 64-slot boundary (slot count = max over cores,
    so all cores share one compile-time schedule); the window's slot list
    is padded to a 128 multiple and split into 128-edge chunks.
  - One batched indirect DMA per 7 windows gathers all source rows of
    those windows' chunks (128 rows per chunk) into SBUF as fp16.  This
    amortizes the ~1us SWDGE descriptor-generation cost per instruction
    that dominated the per-chunk-gather baseline.
  - Per chunk, one tensor_scalar builds the scaled one-hot
    A[e,d] = scale_e * (iota==dst_e) (4x DVE mode; every 3rd chunk runs
    on gpsimd instead to balance engines).  scale folds the 1/cnt mean,
    the 1/T type average, and padding (dst=300 -> zero column).
  - The TensorEngine accumulates S_t^T[f,d] += M^T A into per-type
    column slices of a single [128,512] PSUM tile; a chunk straddling a
    type boundary issues one matmul per 32-aligned partition sub-range
    (legal sub-ranges: start 0/32/64/96 within the quadrant tree).
  - The self term h_w @ W_self_avg needs h_w^T as stationary operand: a
    PE transpose of the SBUF-resident window tile (x / h1 slices stay in
    SBUF all layer, partition-major) produces it in PSUM fp16 - no DMA.
  - Stage 2 per window (software-pipelined one window behind stage 1):
    one Act copy moves the [128,512] S^T block to SBUF fp16, one DVE
    copy moves the self block, then 6 matmuls (4 neighbor + self + K=1
    ones-row bias) accumulate out[d,fo] in one PSUM bank.
  - Layer-1 epilogue fuses square+row-sum (tensor_tensor_reduce), then
    batched sqrt/reciprocal and a per-window scale+relu into the
    resident h1 buffer; one DMA stores h1 (partition-major) and an
    AllGather shares it; layer 2 gathers from the concatenated
    [8*6272,128] table via host-remapped indices.
"""

import numpy as np

N = 50000
E = 600000
F = 128
T = 4
C = 8                      # cores
NPC = N // C               # 6250 destinations per core
WPC = (NPC + 127) // 128   # 49 windows per core
NPP = WPC * 128            # 6272 padded rows per core slice
GBW = 5                    # windows per batched gather instruction
PAD_DST = 300.0            # one-hot miss -> zero column


def _qblocks(lo, hi):
    """Split [lo,hi) (64-aligned) into partition ranges legal for PE
    operands (base partition must be 0, 32, or 64)."""
    out = []
    while lo < hi:
        if lo == 0:
            out.append((0, hi)); lo = hi
        elif lo == 64:
            out.append((64, hi)); lo = hi
        else:
            raise AssertionError(lo)
    return out


def _prep(x, W_self1, W_neigh1, b1, W_self2, W_neigh2, b2, edge_index, edge_type):
    src = np.asarray(edge_index[0], dtype=np.int64)
    dst = np.asarray(edge_index[1], dtype=np.int64)
    et = np.asarray(edge_type, dtype=np.int64)

    cnt = np.bincount(et * N + dst, minlength=T * N).reshape(T, N).astype(np.float32)
    scale_e = (0.25 / np.maximum(cnt[et, dst], 1.0)).astype(np.float32)

    core = dst // NPC
    loc = dst % NPC
    win = loc // 128
    dloc = (loc % 128).astype(np.float32)

    # per (core, window, type) counts; shared slot allocation = max over cores
    key = (core * WPC + win) * T + et
    n_cwt = np.bincount(key, minlength=C * WPC * T).reshape(C, WPC, T)
    S_wt = 64 * ((n_cwt.max(axis=0) + 63) // 64)          # [WPC, T]
    Wslots = S_wt.sum(axis=1)                              # [WPC]
    Kw = (Wslots + 127) // 128                             # chunks per window
    chbase = np.zeros(WPC + 1, dtype=np.int64)
    np.cumsum(Kw, out=chbase[1:])
    NCH = int(chbase[-1])
    tbase = np.zeros((WPC, T), dtype=np.int64)             # slot base of type seg
    tbase[:, 1:] = np.cumsum(S_wt, axis=1)[:, :-1]

    # per-edge slot assignment (order within a (c,w,t) group is arbitrary)
    order = np.lexsort((et, win, core))
    key_s = key[order]
    grp_start_of_key = np.searchsorted(key_s, np.arange(C * WPC * T))
    within = np.arange(E) - grp_start_of_key[key_s]
    sbase_flat = (chbase[:WPC, None] * 128 + tbase).reshape(-1)  # [WPC*T]
    g = sbase_flat[(win * T + et)[order]] + within         # global slot per edge
    p_slot = (g % 128).astype(np.int64)
    ch_slot = (g // 128).astype(np.int64)

    idx1 = np.zeros((C, 128, NCH), dtype=np.int32)
    idx2 = np.zeros((C, 128, NCH), dtype=np.int32)
    dstc = np.full((C, 128, NCH), PAD_DST, dtype=np.float32)
    sclc = np.zeros((C, 128, NCH), dtype=np.float32)

    # indices premultiplied by F: gather tables are addressed as flat 1-D
    # arrays so each descriptor covers a whole per-partition run (>=512B,
    # avoiding the sub-512B descriptor latency penalty)
    src_s = src[order]
    csrc = src_s // NPC
    lsrc = src_s % NPC
    row2 = ((csrc * NPP + (lsrc % 128) * WPC + lsrc // 128) * F).astype(np.int32)
    core_s = core[order]
    idx1[core_s, p_slot, ch_slot] = (src_s * F).astype(np.int32)
    idx2[core_s, p_slot, ch_slot] = row2
    dstc[core_s, p_slot, ch_slot] = dloc[order]
    sclc[core_s, p_slot, ch_slot] = scale_e[order]

    # compile-time matmul schedule per window: (ch, lo, hi, t, start, stop)
    sched = []
    for w in range(WPC):
        blocks = []
        for t in range(T):
            if S_wt[w, t] == 0:
                continue
            s0, s1 = int(tbase[w, t]), int(tbase[w, t] + S_wt[w, t])
            tblocks = []
            for ch in range(int(chbase[w]), int(chbase[w + 1])):
                c0 = (ch - chbase[w]) * 128
                lo, hi = max(s0, c0), min(s1, c0 + 128)
                if lo < hi:
                    for ql, qh in _qblocks(lo - c0, hi - c0):
                        tblocks.append([ch, ql, qh, t, False, False])
            tblocks[0][4] = True
            tblocks[-1][5] = True
            blocks.extend(tuple(b) for b in tblocks)
        sched.append(blocks)

    # weights / bias packs, laid out for one-DMA loads:
    # wpack[p, k*F+f] = W_k[p, f]
    wstack = np.empty((2 * (T + 1), F, F), dtype=np.float16)
    wstack[0:T] = np.asarray(W_neigh1, np.float32).astype(np.float16)
    wstack[T] = np.asarray(W_self1, np.float32).mean(axis=0).astype(np.float16)
    wstack[T + 1:2 * T + 1] = np.asarray(W_neigh2, np.float32).astype(np.float16)
    wstack[2 * T + 1] = np.asarray(W_self2, np.float32).mean(axis=0).astype(np.float16)
    wpack = np.ascontiguousarray(
        wstack.transpose(1, 0, 2).reshape(F, 2 * (T + 1) * F))
    bpack = np.concatenate([
        np.asarray(b1, np.float32).mean(axis=0),
        np.asarray(b2, np.float32).mean(axis=0),
    ]).astype(np.float16)[None, :]

    x16 = np.asarray(x, np.float32).astype(np.float16)
    xpad = np.zeros((C, NPP, F), dtype=np.float16)
    for c in range(C):
        xpad[c, :NPC] = x16[c * NPC:(c + 1) * NPC]
    # partition-major my-slice: xmyT[c, p, w*128+f] = x[c*NPC + w*128 + p, f]
    xmyT = np.ascontiguousarray(
        xpad.reshape(C, WPC, 128, F).transpose(0, 2, 1, 3).reshape(C, 128, NPP))

    return dict(idx1=idx1, idx2=idx2, dstc=dstc, sclc=sclc, wpack=wpack,
                bpack=bpack, x16=x16, xmyT=xmyT, sched=sched, Kw=Kw,
                chbase=chbase, NCH=NCH, S_wt=S_wt)


def _legalize_sync_waits(nc, max_waits=1):
    """The walrus build in this container caps sync-wait commands per
    instruction; hoist excess waits onto NOPs inserted before the
    instruction on the same engine (sequencers execute in order)."""
    from concourse import mybir

    ctr = [0]
    for fn in nc.m.functions:
        for bb in fn.blocks:
            insts = bb.instructions
            if not any(
                i.sync_info is not None and len(i.sync_info.on_wait) > max_waits
                for i in insts
            ):
                continue
            out = []
            for inst in insts:
                si = inst.sync_info
                if si is not None and len(si.on_wait) > max_waits:
                    waits = list(si.on_wait)
                    keep = waits[-max_waits:]
                    hoist = waits[:-max_waits]
                    for i in range(0, len(hoist), max_waits):
                        nop = mybir.InstNoOp(
                            name=f"I-waitsplit-{ctr[0]}", ins=[], outs=[])
                        ctr[0] += 1
                        nop.engine = inst.engine
                        nop.sync_info = mybir.SyncInfo(
                            on_wait=hoist[i:i + max_waits], on_update=[])
                        out.append(nop)
                    inst.sync_info = mybir.SyncInfo(
                        on_wait=keep, on_update=list(si.on_update))
                out.append(inst)
            insts.clear()
            insts.extend(out)


def build_module(prep, legalize=True, n_cores=C):
    import concourse.bass as bass
    import concourse.tile as tile
    from concourse import mybir

    f16, f32, i32 = mybir.dt.float16, mybir.dt.float32, mybir.dt.int32
    Alu = mybir.AluOpType
    Act = mybir.ActivationFunctionType

    NCH = prep["NCH"]
    Kw = prep["Kw"]
    chbase = prep["chbase"]
    sched = prep["sched"]

    nc = bass.Bass(trn_type="TRN2")
    t_x16 = nc.dram_tensor("x16", [N, F], f16, kind="ExternalInput")
    t_xmyT = nc.dram_tensor("xmyT", [128, NPP], f16, kind="ExternalInput")
    t_idx1 = nc.dram_tensor("idx1", [128, NCH], i32, kind="ExternalInput")
    t_idx2 = nc.dram_tensor("idx2", [128, NCH], i32, kind="ExternalInput")
    t_dstc = nc.dram_tensor("dstc", [128, NCH], f32, kind="ExternalInput")
    t_sclc = nc.dram_tensor("sclc", [128, NCH], f32, kind="ExternalInput")
    t_wpack = nc.dram_tensor("wpack", [F, 2 * (T + 1) * F], f16, kind="ExternalInput")
    t_bpack = nc.dram_tensor("bpack", [1, 2 * F], f16, kind="ExternalInput")
    t_out = nc.dram_tensor("out", [128, NPP], f16, kind="ExternalOutput")

    with tile.TileContext(nc, num_cores=n_cores) as tc:
        with tc.tile_pool(name="const", bufs=1) as cpool, \
             tc.tile_pool(name="gath", bufs=3) as gpool, \
             tc.tile_pool(name="onehot", bufs=32) as apool, \
             tc.tile_pool(name="stage2", bufs=2) as spool, \
             tc.tile_pool(name="epi", bufs=2) as epool, \
             tc.tile_pool(name="spsum", bufs=2, space="PSUM") as pspool, \
             tc.tile_pool(name="opsum", bufs=2, space="PSUM") as opool, \
             tc.tile_pool(name="dram", bufs=1, space="DRAM") as dpool:

            # load order matters: idx1 gates the first gather, dstc/sclc gate
            # the one-hot builds, weights gate stage 2 of window 0
            idx1_t = cpool.tile([128, NCH], i32)
            nc.sync.dma_start(out=idx1_t[:], in_=t_idx1[:])
            dstc_t = cpool.tile([128, NCH], f32)
            nc.sync.dma_start(out=dstc_t[:], in_=t_dstc[:])
            sclc_t = cpool.tile([128, NCH], f32)
            nc.sync.dma_start(out=sclc_t[:], in_=t_sclc[:])
            w_sb = cpool.tile([128, 2 * (T + 1) * F], f16)
            nc.sync.dma_start(out=w_sb[:], in_=t_wpack[:])
            b_sb = cpool.tile([1, 2 * F], f16)
            nc.sync.dma_start(out=b_sb[:], in_=t_bpack[:])
            x_myb = cpool.tile([128, NPP], f16)
            nc.sync.dma_start(out=x_myb[:], in_=t_xmyT[:])
            idx2_t = cpool.tile([128, NCH], i32)
            nc.sync.dma_start(out=idx2_t[:], in_=t_idx2[:])
            ones_sb = cpool.tile([1, 128], f16)
            nc.vector.memset(ones_sb[:], 1.0)
            eps_sb = cpool.tile([128, 1], f32)
            nc.vector.memset(eps_sb[:], 1e-24)

            iota_i = cpool.tile([128, 128], i32)
            nc.gpsimd.iota(iota_i[:], pattern=[[1, 128]], base=0, channel_multiplier=0)
            iota_f = cpool.tile([128, 128], f16)
            nc.vector.tensor_copy(out=iota_f[:], in_=iota_i[:])
            iotac_i = cpool.tile([128, 1], i32)
            nc.gpsimd.iota(iotac_i[:], pattern=[[1, 1]], base=0, channel_multiplier=1)
            iotac_f = cpool.tile([128, 1], f32)
            nc.vector.tensor_copy(out=iotac_f[:], in_=iotac_i[:])
            ident = cpool.tile([128, 128], f16)
            nc.vector.tensor_scalar(out=ident[:], in0=iota_f[:],
                                    scalar1=iotac_f[:], scalar2=None,
                                    op0=Alu.is_equal)

            h1buf = cpool.tile([128, NPP], f16)
            obuf0 = cpool.tile([128, NPP], f16)
            obuf1 = cpool.tile([128, NPP], f16)
            ss = cpool.tile([128, WPC], f32)
            nrm = cpool.tile([128, WPC], f32)
            rn = cpool.tile([128, WPC], f32)

            h1_my = dpool.tile([128, NPP], f16)
            h1_all = dpool.tile([C * NPP, F], f16, addr_space="Shared")

            batches = [(w0, min(w0 + GBW, WPC)) for w0 in range(0, WPC, GBW)]

            for layer in (0, 1):
                idx_t = idx1_t if layer == 0 else idx2_t
                src_tbl = t_x16 if layer == 0 else h1_all
                begin_a = 625 if layer == 0 else 784
                selfsrc = x_myb if layer == 0 else h1buf
                obuf = obuf0 if layer == 0 else obuf1
                wofs = layer * (T + 1) * F

                abuild_ctr = 0
                pend = None  # (w, s_sb, s_sbT) awaiting stage 2
                ttr_queue = []  # layer-0 windows awaiting sum-of-squares
                TTR_LAG = 8  # keep DVE several windows behind PE/Act

                def emit_ttr(pw):
                    # row sum-of-squares from the resident obuf slice; lagged
                    # so the DVE never waits on Act's copy
                    sq = epool.tile([128, 128], f16, tag="sq")
                    nc.vector.tensor_tensor_reduce(
                        out=sq[:], in0=obuf[:, pw * 128:(pw + 1) * 128],
                        in1=obuf[:, pw * 128:(pw + 1) * 128],
                        scale=1.0, scalar=0.0, op0=Alu.mult, op1=Alu.add,
                        accum_out=ss[:, pw:pw + 1])

                def stage2(pw, ps_sb, ps_sbT):
                    o_ps = opool.tile([128, 128], f32, space="PSUM", tag="o")
                    first = True
                    for t in range(T):
                        if prep["S_wt"][pw, t] == 0:
                            continue
                        nc.tensor.matmul(
                            out=o_ps[:], lhsT=ps_sb[:, t * F:(t + 1) * F],
                            rhs=w_sb[:, wofs + t * F:wofs + (t + 1) * F],
                            start=first, stop=False)
                        first = False
                    nc.tensor.matmul(
                        out=o_ps[:], lhsT=ps_sbT[:],
                        rhs=w_sb[:, wofs + T * F:wofs + (T + 1) * F],
                        start=first, stop=False)
                    nc.tensor.matmul(
                        out=o_ps[:], lhsT=ones_sb[:],
                        rhs=b_sb[:, layer * F:(layer + 1) * F],
                        start=False, stop=True)
                    nc.scalar.activation(out=obuf[:, pw * 128:(pw + 1) * 128],
                                         in_=o_ps[:], func=Act.Copy)

                def emit_gather(bi):
                    # 2-D reshape with a wide contiguous last dim: the cost
                    # model sizes descriptors from the innermost contiguous
                    # run, so per-row 256B descriptors (2x sub-512B latency
                    # penalty) become full-run ones.  axis=1 makes the index
                    # coefficient 1 (indices are premultiplied by F on the
                    # host).
                    gw0, gw1 = batches[bi]
                    gc0, gc1 = int(chbase[gw0]), int(chbase[gw1])
                    if gc1 == gc0:
                        return None
                    tbl2d = src_tbl[:].rearrange("(a c) b -> a (c b)", a=begin_a)
                    m_t = gpool.tile([128, (gc1 - gc0) * F], f16, tag="m")
                    nc.gpsimd.indirect_dma_start(
                        out=m_t[:], out_offset=None, in_=tbl2d,
                        in_offset=bass.IndirectOffsetOnAxis(
                            ap=idx_t[:, gc0:gc1], axis=1))
                    return m_t

                # prefetch gathers two batches ahead (m pool bufs=3) so the
                # indirect DMA of batch b+2 overlaps batch b's compute
                mtiles = {}
                for bi in range(min(2, len(batches))):
                    mtiles[bi] = emit_gather(bi)

                for bi, (w0, w1) in enumerate(batches):
                    if bi + 2 < len(batches):
                        mtiles[bi + 2] = emit_gather(bi + 2)
                    ch0 = int(chbase[w0])
                    m_t = mtiles.pop(bi)
                    for w in range(w0, w1):
                        # stage 1: per-chunk scaled one-hot + matmul blocks
                        s_all = pspool.tile([128, T * 128], f32, space="PSUM",
                                            tag="sall")
                        s_self = pspool.tile([128, 128], f16, space="PSUM",
                                             tag="sself")
                        a_of_chunk = {}
                        for (ch, lo, hi, t, st, sp) in sched[w]:
                            if ch not in a_of_chunk:
                                a_t = apool.tile([128, 128], f16, tag="a")
                                abuild_ctr += 1
                                nc.vector.tensor_scalar(
                                    out=a_t[:], in0=iota_f[:],
                                    scalar1=dstc_t[:, ch:ch + 1],
                                    scalar2=sclc_t[:, ch:ch + 1],
                                    op0=Alu.is_equal, op1=Alu.mult)
                                a_of_chunk[ch] = a_t
                            a_t = a_of_chunk[ch]
                            mc = (ch - ch0) * F
                            nc.tensor.matmul(
                                out=s_all[:, t * 128:(t + 1) * 128],
                                lhsT=m_t[lo:hi, mc:mc + F],
                                rhs=a_t[lo:hi, :], start=st, stop=sp)
                        nc.tensor.transpose(
                            out=s_self[:], in_=selfsrc[:, w * 128:(w + 1) * 128],
                            identity=ident[:])
                        # copies to SBUF for stage 2 (both on Act: DVE must
                        # stay free of PE-dependent ops or its in-order queue
                        # stalls the a-build pipeline)
                        s_sb = spool.tile([128, T * 128], f16, tag="ssb")
                        nc.scalar.activation(out=s_sb[:], in_=s_all[:],
                                             func=Act.Copy)
                        s_sbT = spool.tile([128, 128], f16, tag="ssbT")
                        nc.gpsimd.tensor_copy(out=s_sbT[:], in_=s_self[:])
                        # stage 2 of the previous window (software pipeline)
                        if pend is not None:
                            stage2(*pend)
                            if layer == 0:
                                ttr_queue.append(pend[0])
                                if len(ttr_queue) > TTR_LAG:
                                    emit_ttr(ttr_queue.pop(0))
                        pend = (w, s_sb, s_sbT)
                if pend is not None:
                    stage2(*pend)
                    if layer == 0:
                        ttr_queue.append(pend[0])
                for pw in ttr_queue:
                    emit_ttr(pw)

                if layer == 0:
                    nc.scalar.activation(out=nrm[:], in_=ss[:],
                                         func=Act.Sqrt, bias=eps_sb[:])
                    nc.vector.reciprocal(out=rn[:], in_=nrm[:])
                    for w in range(WPC):
                        nc.vector.tensor_scalar(
                            out=h1buf[:, w * 128:(w + 1) * 128],
                            in0=obuf0[:, w * 128:(w + 1) * 128],
                            scalar1=rn[:, w:w + 1], scalar2=0.0,
                            op0=Alu.mult, op1=Alu.max)
                    nc.sync.dma_start(out=h1_my[:], in_=h1buf[:])
                    nc.gpsimd.collective_compute(
                        "AllGather",
                        mybir.AluOpType.bypass,
                        replica_groups=[list(range(n_cores))],
                        ins=[h1_my[:]],
                        outs=[h1_all[:]],
                    )
                else:
                    nc.sync.dma_start(out=t_out[:], in_=obuf1[:])

    if legalize:
        _legalize_sync_waits(nc)
    return nc


def _unscramble(arr):
    # arr [128, NPP] fp16 partition-major -> [NPC, F] fp32 row-major
    return (arr.reshape(128, WPC, F).transpose(1, 0, 2)
            .reshape(NPP, F)[:NPC].astype(np.float32))


def kernel(**inputs):
    import sys
    if '/opt/trn_rl_repo' not in sys.path:
        sys.path.insert(0, '/opt/trn_rl_repo')

    prep = _prep(
        inputs["x"], inputs["W_self1"], inputs["W_neigh1"], inputs["b1"],
        inputs["W_self2"], inputs["W_neigh2"], inputs["b2"],
        inputs["edge_index"], inputs["edge_type"])

    nc = build_module(prep, legalize=True, n_cores=C)

    from concourse.bass_utils import run_bass_kernel_spmd
    in_maps = [
        {"x16": prep["x16"], "xmyT": prep["xmyT"][c], "idx1": prep["idx1"][c],
         "idx2": prep["idx2"][c], "dstc": prep["dstc"][c],
         "sclc": prep["sclc"][c], "wpack": prep["wpack"], "bpack": prep["bpack"]}
        for c in range(C)
    ]
    res = run_bass_kernel_spmd(nc, in_maps, core_ids=list(range(C)))

    out = np.empty((N, F), dtype=np.float32)
    for c in range(C):
        out[c * NPC:(c + 1) * NPC] = _unscramble(res.results[c]["out"])
    return out
